# revision 70
# baseline (speedup 1.0000x reference)
"""Distributed RoPE-attention kernel for 8 TRN2 NeuronCores (v3).

Sharding: tensor-parallel over heads (4 heads/core) for QKV+attention;
the attention output (bf16) is AllGather'd per local head (4 gathers,
fired as each head's two s-half units store, so heads 0-2 gather under
the remaining attention compute), then each core computes a 512-column
shard of the final wo projection, accumulating gathered head-blocks in
arrival order. Host concatenates the column shards — no all-reduce.

v4 over v2: per-head collectives (wo-tail stall ~100us -> ~0); bf16
broadcast matmuls and a bf16 1/Z (fp32 matmul is 4 cyc/row on the PE);
the softmax Z-path split DVE/PE (3 wide adds reduce 16 expT slices to
4 partials in wbuf0, the PE's ones-matmul contracts the rest); the
first two attention units' QK+exp slices interleaved into the half-1
q/k projection groups so the scalar engine's exp stream (the attention
pacer at ~21.6us/unit) gets a two-unit head start; per-head rope fired
as soon as its q/k groups finish; the z-chain pops placed at sk0/2/8
of the pass1 windows so the in-order DVE queue never blocks it; wo
weight/activation tiles prefetched during attention; 4-slot wv slab
ring; final store split per (dout, s-chunk).

Layouts are all "transposed" ([feature, seq]) so the PE never needs a
transpose: scoresT = kT.T @ qT, exp on ScalarE (PSUM->SBUF, bf16), PV
uses v as the stationary operand (outT = v.T @ expT), the softmax
denominator comes from a ones-column matmul, and the per-position 1/Z
broadcast is a K=1 outer-product matmul whose issue is deferred into
the next unit's matmul stream (keeps the reciprocal off the PE
critical path).

RoPE runs in an even/odd-permuted head basis (host permutes wq/wk
columns; q.k dot products are permutation invariant), which turns the
pair-swap into two 64-partition SBUF->SBUF DMAs plus three elementwise
DVE ops per head.

Raw bass (no Tile): this container's walrus rejects any instruction
with >1 attached sync-wait, so every dependency is an explicit
standalone wait_ge and semaphores are managed manually via cumulative
counters (one counter per buffer family / producer engine).
"""

import sys

sys.path.insert(0, "/opt/trn_rl_repo")

import numpy as np
import ml_dtypes

import concourse.bass as bass
import concourse.mybir as mybir
from concourse.bass_utils import run_bass_kernel_spmd
from concourse import bass_utils as _bu

_orig_run_command = _bu.run_command


def _patched_run_command(cmd, **kw):
    cmd = ["--enable-ldw-opt=true" if c == "--enable-ldw-opt=false" else c
           for c in cmd]
    return _orig_run_command(cmd, **kw)


_bu.run_command = _patched_run_command

N_CORES = 8
S = 2048
D = 4096
NH = 4            # local heads
HD = 128
NL = 512          # local feature columns
NKB = 32          # 128-row blocks over D
SCALE = 1.0 / float(np.sqrt(HD))

BF16 = mybir.dt.bfloat16
F32 = mybir.dt.float32
AF = mybir.ActivationFunctionType
ALU = mybir.AluOpType

ENGINES = ("sync", "tensor", "scalar", "vector", "gpsimd")
LAST_SCHED = None


class Sched:
    """Per-engine straight-line programs with cumulative-counter sems."""

    def __init__(self):
        self.prog = {e: [] for e in ENGINES}
        self.count = {}

    def emit(self, eng, fn, deps=(), inc=None, amt=1):
        dd = {}
        for sem, thr in deps:
            if thr is not None and thr > dd.get(sem, -1):
                dd[sem] = thr
        tick = None
        if inc is not None:
            tick = self.count.get(inc, 0) + amt
            self.count[inc] = tick
        self.prog[eng].append((fn, sorted(dd.items()), inc, amt))
        return tick

    def run(self, eng_name, eng, sems):
        observed = {}
        for fn, deps, inc, amt in self.prog[eng_name]:
            for sem, thr in deps:
                if observed.get(sem, 0) < thr:
                    eng.wait_ge(sems[sem], thr)
                    observed[sem] = thr
            inst = fn(eng)
            if inc is not None:
                inst.then_inc(sems[inc], amt)


def build():
    nc = bass.Bass(num_devices=N_CORES, dynamic_dma_scratch_size=64)

    xT_ext = nc.declare_dram_parameter("xT", [D, S], BF16, isOutput=False)
    wq_ext = nc.declare_dram_parameter("wq", [D, NL], BF16, isOutput=False)
    wk_ext = nc.declare_dram_parameter("wk", [D, NL], BF16, isOutput=False)
    wv_ext = nc.declare_dram_parameter("wv", [D, NL], BF16, isOutput=False)
    wo_ext = nc.declare_dram_parameter("wo", [D, NL], BF16, isOutput=False)
    cosd_ext = nc.declare_dram_parameter("cosd", [HD, S], BF16, isOutput=False)
    sins_ext = nc.declare_dram_parameter("sins", [HD, S], BF16, isOutput=False)
    out_ext = nc.declare_dram_parameter("out", [NL, S], F32, isOutput=True)

    cc_in = nc.dram_tensor("cc_in", [NL, S], BF16)
    cc_out = [
        nc.dram_tensor(f"cc_out{i}", [N_CORES * 128, S], BF16, addr_space="Shared")
        for i in range(NH)
    ]

    sem_names = [
        "xa", "xb", "wb0", "wb1", "wb2", "cs", "swp",
        "ast0", "ast1", "ast2", "ast3",
        "sl0", "sl1", "sl2", "sl3",
        "wl0", "wl1", "wl2", "wl3", "wl4", "wl5", "wl6", "wl7",
        "af0", "af1", "af2", "af3", "af4", "af5", "af6", "af7",
        "yst", "pe", "act", "dve", "cc",
    ]
    # (slab now has 4 slots; sl0..sl3 already declared)

    import contextlib

    with contextlib.ExitStack() as ctx:
        def sb(name, shape, dt):
            return ctx.enter_context(nc.sbuf_tensor(name, shape, dt))

        arenaA = sb("arenaA", [128, 32 * 1024], BF16)   # x half0 -> expT slabs
        arenaB = sb("arenaB", [128, 32 * 1024], BF16)   # x half1 -> afbuf/y_sb
        wbuf = [sb(f"wbuf{i}", [128, NKB * 128], BF16) for i in range(3)]
        slab = sb("slab", [128, 4 * 512], BF16)         # wv stream tiles
        qT_sb = sb("qT_sb", [128, NH * S], BF16)
        kT_sb = sb("kT_sb", [128, NH * S], BF16)
        v_sb = sb("v_sb", [128, 16 * 512], BF16)
        cosd_sb = sb("cosd_sb", [128, S], BF16)
        sins_sb = sb("sins_sb", [128, S], BF16)
        recip_sb = sb("recip", [1, 1024], F32)
        ones_col = sb("onesc", [128, 1], BF16)
        ones_row = sb("onesr", [1, 128], BF16)

        # aliases (temporal reuse, enforced by the schedule):
        qsw = wbuf[0][:, 0:2048]       # rope swap scratch (post q/k groups)
        t1 = wbuf[0][:, 2048:4096]
        t2 = wbuf[1][:, 0:2048]
        # attention-time scratch in arenaB's tail (x half1 dead by then)
        stg = [arenaB[:, 19456 + i * 1024: 19456 + (i + 1) * 1024]
               for i in range(4)]
        recip_bf = arenaB[0:1, 23552:24576]
        # attention normalize scratch aliases rope scratch (dead post-rope;
        # DVE program order separates the uses)
        bc_sb = arenaB[:, 17408:19456].bitcast(F32)  # [128, 1024] f32
        zacc = arenaB[:, 16384:17408]                # [128, 1024] bf16
        woslab = wbuf[2][:, 0:4096]    # 8 x [128,512] wo weight tiles
        afbuf = ([arenaB[:, i * 1024:(i + 1) * 1024] for i in range(4)] +
                 [arenaB[:, 12288 + i * 1024: 12288 + (i + 1) * 1024]
                  for i in range(4)])  # wo rhs, 4 slots per half
        y_sb = arenaB[:, 4096:12288].bitcast(F32)       # [128, 4096] f32

        pairs = [ctx.enter_context(nc.psum_tensor(f"pair{i}", [128, 1024], F32))
                 for i in range(4)]

        sems = {n: ctx.enter_context(nc.semaphore(n)) for n in sem_names}

        sch = Sched()
        global LAST_SCHED
        LAST_SCHED = sch
        _schedule(sch, locals())

        with nc.Block() as block:

            @block.sync
            def _(eng):
                sch.run("sync", eng, sems)

            @block.tensor
            def _(eng):
                sch.run("tensor", eng, sems)

            @block.scalar
            def _(eng):
                sch.run("scalar", eng, sems)

            @block.vector
            def _(eng):
                with nc.allow_low_precision(
                        reason="1/Z kept in bf16 for the broadcast matmul"):
                    sch.run("vector", eng, sems)

            @block.gpsimd
            def _(eng):
                sch.run("gpsimd", eng, sems)

    return nc


def _schedule(sch, env):
    def g(n):
        return env[n]

    xT_ext, wq_ext, wk_ext, wv_ext, wo_ext = (
        g("xT_ext"), g("wq_ext"), g("wk_ext"), g("wv_ext"), g("wo_ext"))
    cosd_ext, sins_ext, out_ext = g("cosd_ext"), g("sins_ext"), g("out_ext")
    cc_in, cc_out = g("cc_in"), g("cc_out")
    arenaA, arenaB, wbuf, slab = g("arenaA"), g("arenaB"), g("wbuf"), g("slab")
    qT_sb, kT_sb, v_sb = g("qT_sb"), g("kT_sb"), g("v_sb")
    cosd_sb, sins_sb = g("cosd_sb"), g("sins_sb")
    qsw, t1, t2 = g("qsw"), g("t1"), g("t2")
    stg, recip_sb, bc_sb = g("stg"), g("recip_sb"), g("bc_sb")
    recip_bf = g("recip_bf")
    zacc = g("zacc")
    woslab = g("woslab")
    ones_col, ones_row = g("ones_col"), g("ones_row")
    afbuf, y_sb = g("afbuf"), g("y_sb")
    pairs = g("pairs")
    arenas = [arenaA, arenaB]

    E = sch.emit

    def dma(out_ap, in_ap):
        return lambda eng: eng.dma_start(out=out_ap, in_=in_ap)

    # ---------------- SP: x loads (both halves up front) ----------------
    xsem = {}

    def emit_x_load(half, j):
        name = "xa" if half == 0 else "xb"
        xsem[half] = E("sync", dma(
            arenas[half][:, j * 8192:(j + 1) * 8192]
            .rearrange("p (kb s) -> p kb s", kb=8),
            xT_ext[j * 1024:(j + 1) * 1024, half * 1024:(half + 1) * 1024]
            .rearrange("(kb p) s -> p kb s", p=128)),
            inc=name, amt=16)

    # wb0 first so group 0 can start as soon as the first x quarter lands
    E("vector", lambda eng: eng.memset(ones_col[:], 1.0), inc="dve")
    dve_ones = E("vector", lambda eng: eng.memset(ones_row[:], 1.0), inc="dve")

    # ---------------- projections ----------------
    bank_war = {}      # (pair_idx, colhalf) -> act tick of last reader
    evac_tick = {}     # ("q"/"k", n, half) -> act tick
    wgrp = {}          # qk group idx -> pe tick of its last matmul
    vk_tick = {}       # (half, k) -> pe tick (for slab WAR)
    v_end = {}
    wb_tick = {}
    sl_count = {i: 0 for i in range(4)}

    GL1 = [0, 4, 1, 5, 2, 6, 3, 7]   # half-1 group order: q0 k0 q1 k1 ...

    def emit_qk_weight_dma(gg):
        half = gg // 8
        gl = GL1[gg - 8] if half == 1 else gg % 8
        t, n = ("q", gl) if gl < 4 else ("k", gl - 4)
        wext = wq_ext if t == "q" else wk_ext
        slot = gg % 3
        deps = []
        if wgrp.get(gg - 3) is not None:
            deps.append(("pe", wgrp[gg - 3]))
        wb_tick[gg] = E("sync", dma(
            wbuf[slot][:].rearrange("p (kb c) -> p kb c", kb=NKB),
            wext[:, n * 128:(n + 1) * 128].rearrange("(kb p) c -> p kb c", p=128)),
            deps=deps, inc=f"wb{slot}", amt=16)

    emit_x_load(0, 0)
    emit_qk_weight_dma(0)
    emit_qk_weight_dma(1)
    emit_x_load(0, 1)
    emit_x_load(0, 2)
    emit_x_load(0, 3)
    E("sync", dma(cosd_sb[:], cosd_ext[:]), inc="cs", amt=16)
    CS_ALL = E("sync", dma(sins_sb[:], sins_ext[:]), inc="cs", amt=16)
    preload_slabs = True

    def emit_one_qk_group(gg, half, gl, pidx):
        t, n = ("q", gl) if gl < 4 else ("k", gl - 4)
        slot = gg % 3
        pair = pairs[pidx]
        xname = "xa" if half == 0 else "xb"
        deps = [(f"wb{slot}", wb_tick[gg])]
        if gg >= 4:
            deps.append((xname, 64))
        else:
            deps.append((xname, 16))
        for chf in range(2):
            if bank_war.get((pidx, chf)) is not None:
                deps.append(("act", bank_war[(pidx, chf)]))
        tick = None
        for k in range(NKB):
            kdeps = ()
            if k == 0:
                kdeps = deps
            elif gg < 4 and k % 8 == 0:
                kdeps = [(xname, 16 * (k // 8 + 1))]
            for sc in range(2):
                last = (k == NKB - 1) and (sc == 1)

                def mm(eng, k=k, sc=sc, pair=pair, slot=slot, half=half):
                    return eng.matmul(
                        pair[:, sc * 512:(sc + 1) * 512],
                        wbuf[slot][:, k * 128:(k + 1) * 128],
                        arenas[half][:, k * 1024 + sc * 512:
                                     k * 1024 + (sc + 1) * 512],
                        start=(k == 0), stop=(k == NKB - 1))

                tick = E("tensor", mm,
                         deps=kdeps if sc == 0 else (),
                         inc="pe" if last else None)
        wgrp[gg] = tick
        dst = qT_sb if t == "q" else kT_sb

        def evac(eng, dst=dst, n=n, half=half, pair=pair):
            return eng.copy(
                dst[:, n * S + half * 1024: n * S + (half + 1) * 1024],
                pair[:, 0:1024])

        a = E("scalar", evac, deps=[("pe", tick)], inc="act")
        bank_war[(pidx, 0)] = a
        bank_war[(pidx, 1)] = a
        evac_tick[(t, n, half)] = a

    def emit_qk_groups0():
        for gl in range(8):
            emit_one_qk_group(gl, 0, gl, gl % 2)
            if gl + 2 <= 7:
                emit_qk_weight_dma(gl + 2)
            if gl in (1, 3, 5, 7):
                emit_x_load(1, (gl - 1) // 2)

    def emit_qk_groups_h1():
        # half-1 q/k groups in q0,k0,q1,k1,... order on psum pairs 2/3,
        # with per-head rope and the first two attention units' QK+exp
        # slices interleaved (their exps hide under the projection PE work)
        for p in range(8):
            gg = 8 + p
            emit_one_qk_group(gg, 1, GL1[p], 2 + p % 2)
            if gg + 3 <= 15:
                emit_qk_weight_dma(gg + 3)
            if p % 2 == 1:
                hh = p // 2
                emit_rope("q", qT_sb, hh)
                emit_rope("k", kT_sb, hh)
            if p == 3:
                emit_pass1(0, range(0, 6))
            elif p == 4:
                emit_pass1(0, range(6, 12))
            elif p == 5:
                emit_pass1(0, range(12, 16))
            elif p == 6:
                emit_pass1(1, range(0, 8))
            elif p == 7:
                emit_pass1(1, range(8, 16))

    def emit_slab(half, k):
        slot = k % 4
        war = vk_tick.get((half, k - 4))
        if war is None and half == 1:
            war = vk_tick.get((0, k + NKB - 4))
        deps = [("pe", war)] if war is not None else []
        sl_count[slot] += 16
        E("sync", dma(
            slab[:, slot * 512:(slot + 1) * 512],
            wv_ext[k * 128:(k + 1) * 128, :]),
          deps=deps, inc=f"sl{slot}", amt=16)

    def emit_v_groups(half):
        tick = None
        for k in range(NKB):
            slot = k % 4
            deps = [(f"sl{slot}", 16 * (half * 8 + k // 4 + 1))]
            if k == 0:
                for pidx in range(4):
                    for chf in range(2):
                        if bank_war.get((pidx, chf)) is not None:
                            deps.append(("act", bank_war[(pidx, chf)]))
            for st in range(8):
                last = st == 7

                def mmv(eng, k=k, st=st, half=half, slot=slot):
                    return eng.matmul(
                        pairs[st // 2][:, (st % 2) * 512:(st % 2 + 1) * 512],
                        arenas[half][:, k * 1024 + st * 128:
                                     k * 1024 + st * 128 + 128],
                        slab[:, slot * 512:(slot + 1) * 512],
                        start=(k == 0), stop=(k == NKB - 1))

                tick = E("tensor", mmv, deps=deps if st == 0 else (),
                         inc="pe" if last else None)
            vk_tick[(half, k)] = tick
            if k + 4 < NKB:
                emit_slab(half, k + 4)
            if half == 0 and k < 2:
                emit_qk_weight_dma(8 + k)
            if half == 0 and k == 2:
                emit_qk_weight_dma(10)
        v_end[half] = tick
        for st in range(8):
            stg_idx = half * 8 + st

            def evacv(eng, stg_idx=stg_idx, st=st):
                return eng.copy(
                    v_sb[:, stg_idx * 512:(stg_idx + 1) * 512],
                    pairs[st // 2][:, (st % 2) * 512:(st % 2 + 1) * 512])

            evdeps = [("pe", v_end[half])]
            if half == 1:
                evdeps.append(("dve", rope_last))  # v_sb rope-scratch WAR
            a = E("scalar", evacv, deps=evdeps, inc="act")
            bank_war[(st // 2, st % 2)] = a

    # ---------------- RoPE (in-place, v_sb tail scratch) ----------------
    swp = 0
    prev_sw = None
    rope_last = None
    rope_done = {}
    rp_t1 = v_sb[:, 4096:6144]   # v(h1) region: free until v(h1) evacs

    def emit_rope(t, tsb, h):
        nonlocal swp, prev_sw, rope_last
        c0 = h * S
        d0 = [("act", evac_tick[(t, h, 0)]), ("act", evac_tick[(t, h, 1)])]
        dsw = d0 + ([("dve", prev_sw)] if prev_sw is not None else [])
        swp = E("scalar", dma(v_sb[0:64, 6144:8192],
                              tsb[64:128, c0:c0 + S]),
                deps=dsw, inc="swp", amt=16)
        swp = E("scalar", dma(v_sb[64:128, 6144:8192],
                              tsb[0:64, c0:c0 + S]), inc="swp", amt=16)

        def f_t1(eng, tsb=tsb, c0=c0):
            return eng.tensor_mul(rp_t1, tsb[:, c0:c0 + S], cosd_sb[:])

        E("vector", f_t1, deps=d0 + [("cs", CS_ALL)], inc="dve")

        def f_t2(eng, tsb=tsb, c0=c0):
            return eng.tensor_mul(tsb[:, c0:c0 + S], v_sb[:, 6144:8192],
                                  sins_sb[:])

        prev_sw = E("vector", f_t2, deps=[("swp", swp)], inc="dve")

        def f_add(eng, tsb=tsb, c0=c0):
            return eng.tensor_add(tsb[:, c0:c0 + S], tsb[:, c0:c0 + S],
                                  rp_t1)

        rope_last = rope_done[(t, h)] = E("vector", f_add, inc="dve")

    # ------------- attention: 8 half-units (head-major) -------------
    # dunit d = h*2 + qp covers head h, s-half qp (two sq quarters).
    # expT slab (d%2) = arenaA[:, (d%2)*16384 : +16384] as [16 sk][1024].
    # AllGather is per local head (4 gathers): gather(h) fires as soon as
    # units 2h, 2h+1 have stored, so gathers h0-h2 hide under attention.
    # wo kb enumerates (head, core): kb = h*8 + c -> gathered rows
    # cc_out[h][c*128:...], weight rows wo_ext[(c*4 + h)*128:...].
    wo_kb_tick = {}
    af = {i: 0 for i in range(8)}
    wl_count = {i: 0 for i in range(8)}
    af_loaded = set()
    wl_loaded = set()

    def emit_afbuf_load(half, kb, engine="sync"):
        if (half, kb) in af_loaded:
            return
        af_loaded.add((half, kb))
        h, c = kb // 8, kb % 8
        aslot = half * 4 + kb % 4
        war = wo_kb_tick.get((half, kb - 4))
        deps = [("cc", h + 1)]
        if war is not None:
            deps.append(("pe", war))
        af[aslot] += 16
        E(engine, dma(
            afbuf[aslot],
            cc_out[h][c * 128:(c + 1) * 128,
                      half * 1024:(half + 1) * 1024]),
          deps=deps, inc=f"af{aslot}", amt=16)

    def emit_woslab_load(half, kb, engine="sync"):
        if (half, kb) in wl_loaded:
            return
        wl_loaded.add((half, kb))
        h, c = kb // 8, kb % 8
        wslot = kb % 8
        sdeps = [("pe", wgrp[14])]   # wbuf[2] WAR (last qk reader)
        swar = wo_kb_tick.get((half, kb - 8))
        if swar is None and half == 1:
            swar = wo_kb_tick.get((0, kb + NKB - 8))
        if swar is not None:
            sdeps.append(("pe", swar))
        wl_count[wslot] += 16
        wrow = (c * NH + h) * 128
        E(engine, dma(
            woslab[:, wslot * 512:(wslot + 1) * 512],
            wo_ext[wrow:wrow + 128, :]),
          deps=sdeps, inc=f"wl{wslot}", amt=16)

    def emit_wo_loads(half, kb, engine="sync"):
        emit_afbuf_load(half, kb, engine)
        emit_woslab_load(half, kb, engine)

    exp_last = {}
    pv_read_end = {}
    state = {"stt": None, "recip": None, "bc": None, "bcast": None,
             "zmm": None, "adds": {}}
    stg_store = {}
    store_tick = {}
    ps_o_pair = {}
    ast = {0: 0, 1: 0, 2: 0, 3: 0}
    pending_zr = []
    pending_bc = []
    pending_adds = []
    adds_l1 = {}

    def finish_unit(d, bcast_tick):
        h, qp = d // 2, d % 2
        state["bc"] = E(
            "vector",
            lambda eng: eng.tensor_copy(bc_sb[:], pairs[3][:, 0:1024]),
            deps=[("pe", bcast_tick)], inc="dve")
        slot = d % 4
        sdeps = []
        if slot in stg_store:
            sdeps.append(stg_store[slot])

        def f_stt(eng, slot=slot, d=d):
            return eng.scalar_tensor_tensor(
                stg[slot][:], pairs[ps_o_pair[d]][:, 0:1024], 1.0, bc_sb[:],
                ALU.mult, ALU.mult)

        state["stt"] = E("vector", f_stt, deps=sdeps, inc="dve")

        sem = f"ast{slot}"
        ast[slot] += 16
        E("sync", dma(
            cc_in[h * 128:(h + 1) * 128, qp * 1024:(qp + 1) * 1024],
            stg[slot][:]),
            deps=[("dve", state["stt"])], inc=sem, amt=16)
        stg_store[slot] = (sem, ast[slot])
        store_tick[d] = (sem, ast[slot])

    def make_zr(d):
        def emit_zr():
            dps = [("dve", state["adds"][d]), ("dve", dve_ones)]
            if state["recip"] is not None:
                dps.append(("dve", state["recip"]))  # ps_z WAR
            if state["bc"] is not None:
                dps.append(("dve", state["bc"]))  # pairs[3] WAR vs bc copy
            for chf in range(2):
                bw = bank_war.get((3, chf))
                if bw is not None:
                    dps.append(("act", bw))  # pairs[3] WAR vs v(h1) evacs
            # Z = ones.T @ partials: contract the remaining 4 partial
            # slices on the PE (8 accumulating mms) instead of more DVE adds
            zmm = None
            for ch in range(4):
                for zc in range(2):
                    def fz(eng, ch=ch, zc=zc):
                        return eng.matmul(
                            pairs[3][0:1, zc * 512:(zc + 1) * 512], ones_col[:],
                            wbuf[0][:, ch * 1024 + zc * 512:
                                    ch * 1024 + (zc + 1) * 512],
                            start=(ch == 0), stop=(ch == 3))
                    last = ch == 3 and zc == 1
                    zmm = E("tensor", fz,
                            deps=dps if (ch == 0 and zc == 0) else (),
                            inc="pe" if last else None)
            state["zmm"] = zmm
            rdeps = [("pe", zmm)]
            if state["bcast"] is not None:
                rdeps.append(("pe", state["bcast"]))  # recip_bf WAR
            state["recip"] = E(
                "vector",
                lambda eng: eng.reciprocal(recip_bf[:], pairs[3][0:1, 0:1024]),
                deps=rdeps, inc="dve")
        return emit_zr

    def make_bcast(d):
        def emit_bcast():
            dps = [("dve", state["recip"])]
            if state["bc"] is not None:
                dps.append(("dve", state["bc"]))
            for chf in range(2):
                bw = bank_war.get((3, chf))
                if bw is not None:
                    dps.append(("act", bw))  # pairs[3] WAR vs v(h1) evacs
            bt = None
            for zc in range(2):
                def fb(eng, zc=zc):
                    return eng.matmul(
                        pairs[3][:, zc * 512:(zc + 1) * 512], ones_row[:],
                        recip_bf[:, zc * 512:(zc + 1) * 512],
                        start=True, stop=True)
                bt = E("tensor", fb, deps=dps if zc == 0 else (),
                       inc="pe" if zc == 1 else None)
            state["bcast"] = bt
            finish_unit(d, bt)
        return emit_bcast

    def emit_pass1(d, sks=None):
        h, qp = d // 2, d % 2
        base = (d % 2) * 16384
        if sks is None:
            sks = range(16)
        for sk in sks:
            pidx = sk % 2
            pair = pairs[pidx]
            deps = [("dve", rope_done[("q", h)]), ("dve", rope_done[("k", h)])]
            for chf in range(2):
                if bank_war.get((pidx, chf)) is not None:
                    deps.append(("act", bank_war[(pidx, chf)]))
            tick = None
            for qi in range(2):

                def mm1(eng, pair=pair, h=h, sk=sk, qp=qp, qi=qi):
                    return eng.matmul(
                        pair[:, qi * 512:(qi + 1) * 512],
                        kT_sb[:, h * S + sk * 128: h * S + sk * 128 + 128],
                        qT_sb[:, h * S + qp * 1024 + qi * 512:
                              h * S + qp * 1024 + (qi + 1) * 512],
                        start=True, stop=True)

                tick = E("tensor", mm1, deps=deps if qi == 0 else (),
                         inc="pe" if qi == 1 else None)

            if sk == 0 and pending_zr:
                pending_zr.pop(0)()
            if pending_adds and ((sk == 2 and d >= 2)
                                 or (sk == 10 and d == 1)):
                # d==1 defers to its second slice chunk: group 15 (the last
                # wbuf[0] weight reader) must be emitted first
                pending_adds.pop(0)()
            if sk == 8 and pending_bc:
                pending_bc.pop(0)()

            edeps = [("pe", tick)]
            if d >= 2 and sk == 0:
                edeps.append(("pe", pv_read_end[d - 2]))
                edeps.append(("dve", adds_l1[d - 2]))
            if d < 2 and sk == 0:
                edeps.append(("pe", P_H0_END))  # arenaA WAR vs half0 x

            def f_exp(eng, pair=pair, base=base, sk=sk):
                return eng.activation(
                    arenaA[:, base + sk * 1024: base + (sk + 1) * 1024],
                    pair[:, 0:1024], AF.Exp, scale=SCALE)

            a = E("scalar", f_exp, deps=edeps, inc="act")
            exp_last[d] = a
            bank_war[(pidx, 0)] = a
            bank_war[(pidx, 1)] = a
        if 15 in sks:
            pending_adds.append(make_adds(d))

    def emit_pass2(d, opair=2):
        h, qp = d // 2, d % 2
        base = (d % 2) * 16384
        deps = [("act", exp_last[d])]
        if opair == 2 and state["stt"] is not None:
            deps.append(("dve", state["stt"]))
        # pair WAR + v_sb RAW vs the v(h1) evacs: units 0/1's exps precede
        # the v evacs on the act queue, so program order no longer covers
        # it. (3,1) is the last v evac; stale-but-harmless for later units.
        bw = bank_war.get((3, 1))
        if bw is not None:
            deps.append(("act", bw))
        for chf in range(2):
            bw = bank_war.get((opair, chf))
            if bw is not None:
                deps.append(("act", bw))
        tick = None
        for sk in range(16):
            for qi in range(2):

                def mpv(eng, sk=sk, h=h, base=base, qi=qi, opair=opair):
                    return eng.matmul(
                        pairs[opair][:, qi * 512:(qi + 1) * 512],
                        v_sb[:, sk * 512 + h * 128: sk * 512 + h * 128 + 128],
                        arenaA[:, base + sk * 1024 + qi * 512:
                               base + sk * 1024 + (qi + 1) * 512],
                        start=(sk == 0), stop=(sk == 15))

                tick = E("tensor", mpv,
                         deps=deps if (sk == 0 and qi == 0) else (),
                         inc="pe" if (sk == 15 and qi == 1) else None)
        pv_read_end[d] = tick
        ps_o_pair[d] = opair

        pending_zr.append(make_zr(d))
        pending_bc.append(make_bcast(d))

    def make_adds(d):
        # reduce 16 expT slices to 4 partials in wbuf[0] (dead after the
        # last qk group; the PE's zmm contracts the rest). Popped into the
        # NEXT pass1 window so the DVE queue never blocks the z-chain.
        base = (d % 2) * 16384

        def emit_adds():
            adeps = [("act", exp_last[d]), ("pe", wgrp[15])]
            if state["zmm"] is not None:
                adeps.append(("pe", state["zmm"]))  # scratch WAR vs zmm reads

            def fa(eng, q, acc, base=base):
                sl = arenaA[:, base + q * 4096:base + (q + 1) * 4096]
                if not acc:
                    return eng.tensor_add(
                        wbuf[0][:, 0:4096],
                        sl, arenaA[:, base + 4096 * (q + 1):
                                   base + 4096 * (q + 2)])
                return eng.tensor_add(wbuf[0][:, 0:4096],
                                      wbuf[0][:, 0:4096], sl)

            E("vector", lambda eng: fa(eng, 0, False), deps=adeps)
            E("vector", lambda eng: fa(eng, 2, True))
            tick = E("vector", lambda eng: fa(eng, 3, True), inc="dve")
            adds_l1[d] = state["adds"][d] = tick
        return emit_adds

    # PE order: qk(h0), v(h0), qk(h1)+rope+pass1(0,1), v(h1), attention
    for k in range(4):
        emit_slab(0, k)
    emit_qk_groups0()
    emit_v_groups(0)
    P_H0_END = v_end[0]
    for k in range(4):
        emit_slab(1, k)
    emit_qk_groups_h1()
    emit_v_groups(1)

    for d in range(2, 8):
        emit_pass2(d - 2)
        emit_pass1(d)
        if d == 3:
            # woslab tiles for head 0 prefetch during early attention
            # (wbuf[2] WAR only — no collective dependency)
            for kb_pre in range(8):
                emit_woslab_load(0, kb_pre)
    # afbuf preloads sit on the sync queue BEFORE the last units' cc_in
    # stores; their cc(1) dep is satisfied mid-attention so they stream in
    # well before the wo matmuls need them
    for kb_pre in range(4):
        emit_afbuf_load(0, kb_pre)
    emit_pass2(6)             # PV(6); queues zr6/bc6
    pending_zr.pop(0)()       # zmm(6)+recip(6): reads ztree(6) before L1(7)
    pending_adds.pop(0)()     # tree(7), gated on zmm(6) via ztree WAR
    emit_pass2(7, opair=0)    # PV(7) -> pair0, overlaps unit 6's chain
    pending_bc.pop(0)()       # bcast(6)+stt(6)
    pending_zr.pop(0)()       # zmm(7)+recip(7)
    pending_bc.pop(0)()       # bcast(7)+stt(7)

    for h in range(NH):

        def f_ag(eng, h=h):
            return eng.collective_compute(
                "AllGather", ALU.bypass,
                replica_groups=[list(range(N_CORES))],
                ins=[cc_in[h * 128:(h + 1) * 128, :].opt()],
                outs=[cc_out[h][:].opt()])

        E("gpsimd", f_ag,
          deps=[store_tick[2 * h], store_tick[2 * h + 1]],
          inc="cc")

    # ---------------- wo projection ----------------
    y_evac_by_dout = {}
    for half in range(2):
        for kb in range(NKB):
            slot = kb % 4
            aslot = half * 4 + slot
            wslot = kb % 8
            emit_wo_loads(half, kb)

            mmdeps = [(f"af{aslot}", af[aslot]), (f"wl{wslot}", wl_count[wslot])]
            if kb == 0 and half == 0:
                mmdeps.append(("act", exp_last[7]))
                mmdeps.append(("dve", state["stt"]))
                mmdeps.append(("dve", state["recip"]))
            tick = None
            for dout in range(4):
                for c2 in range(2):
                    dd = mmdeps if (dout == 0 and c2 == 0) else []
                    if kb == 0 and half == 1 and c2 == 0:
                        # pairs[dout] WAR: only needs half-0's evacs of
                        # this dout, not the full evac+store tail
                        dd = list(dd) + [("act", y_evac_by_dout[dout])]

                    def mmo(eng, kb=kb, dout=dout, c2=c2,
                            aslot=aslot, wslot=wslot):
                        return eng.matmul(
                            pairs[dout][:, c2 * 512:(c2 + 1) * 512],
                            woslab[:, wslot * 512 + dout * 128:
                                   wslot * 512 + dout * 128 + 128],
                            afbuf[aslot][:, c2 * 512:(c2 + 1) * 512],
                            start=(kb == 0), stop=(kb == NKB - 1))

                    tick = E("tensor", mmo,
                             deps=dd,
                             inc="pe" if (dout == 3 and c2 == 1) else None)
            wo_kb_tick[(half, kb)] = tick

        wo_end = wo_kb_tick[(half, NKB - 1)]
        if half == 0:
            for kb_pre in range(4):
                emit_wo_loads(1, kb_pre, engine="scalar")
        for c2 in range(2):
            for dout in range(4):
                deps = [("pe", wo_end)]
                if half == 1:
                    deps.append(("yst", 16 * (c2 * 4 + dout + 1)))

                def evy(eng, dout=dout, c2=c2):
                    return eng.copy(
                        y_sb[:, (dout * 2 + c2) * 512:(dout * 2 + c2 + 1) * 512],
                        pairs[dout][:, c2 * 512:(c2 + 1) * 512])

                y_evac_last = E("scalar", evy, deps=deps, inc="act")
                if c2 == 1:
                    y_evac_by_dout[dout] = y_evac_last

                cbase = half * 1024 + c2 * 512
                E("sync", dma(
                    out_ext[dout * 128:(dout + 1) * 128, cbase:cbase + 512],
                    y_sb[:, (dout * 2 + c2) * 512:(dout * 2 + c2 + 1) * 512]),
                    deps=[("act", y_evac_last)], inc="yst", amt=16)

    E("sync", lambda eng: None, deps=[("yst", 256)])


# ======================= host side =======================

_NC_CACHE = None


def _get_nc():
    global _NC_CACHE
    if _NC_CACHE is None:
        _NC_CACHE = build()
    return _NC_CACHE


def _prep_inputs(x, freqs_cos, freqs_sin, wq, wk, wv, wo):
    bf = ml_dtypes.bfloat16
    x2 = np.asarray(x, np.float32).reshape(S, D)
    xT = np.ascontiguousarray(x2.T).astype(bf)
    perm = np.concatenate([np.arange(0, HD, 2), np.arange(1, HD, 2)])
    cos = np.asarray(freqs_cos, np.float32)
    sin = np.asarray(freqs_sin, np.float32)
    cosd = np.concatenate([cos.T, cos.T], axis=0).astype(bf)
    sins = np.concatenate([-sin.T, sin.T], axis=0).astype(bf)

    in_maps = []
    for c in range(N_CORES):
        cols_qk = np.concatenate([c * NL + h * HD + perm for h in range(NH)])
        cols_n = np.arange(c * NL, (c + 1) * NL)
        in_maps.append({
            "xT": xT,
            "wq": np.ascontiguousarray(np.asarray(wq, np.float32)[:, cols_qk]).astype(bf),
            "wk": np.ascontiguousarray(np.asarray(wk, np.float32)[:, cols_qk]).astype(bf),
            "wv": np.ascontiguousarray(np.asarray(wv, np.float32)[:, cols_n]).astype(bf),
            "wo": np.ascontiguousarray(np.asarray(wo, np.float32)[:, cols_n]).astype(bf),
            "cosd": cosd,
            "sins": sins,
        })
    return in_maps


def run(inputs, trace=False, **kw):
    nc = _get_nc()
    in_maps = _prep_inputs(**inputs)
    res = run_bass_kernel_spmd(nc, in_maps, core_ids=list(range(N_CORES)),
                               trace=trace, **kw)
    yT = np.concatenate([np.asarray(res.results[c]["out"], np.float32)
                         for c in range(N_CORES)], axis=0)
    out = np.ascontiguousarray(yT.T).reshape(1, S, D).astype(np.float32)
    return out, res


def kernel(x, freqs_cos, freqs_sin, wq, wk, wv, wo):
    out, _ = run(dict(x=x, freqs_cos=freqs_cos, freqs_sin=freqs_sin,
                      wq=wq, wk=wk, wv=wv, wo=wo))
    return out



# revision 73
# speedup vs baseline: 1.0153x; 1.0153x over previous
"""Distributed RoPE-attention kernel for 8 TRN2 NeuronCores (v3).

Sharding: tensor-parallel over heads (4 heads/core) for QKV+attention;
the attention output (bf16) is AllGather'd per local head (4 gathers,
fired as each head's two s-half units store, so heads 0-2 gather under
the remaining attention compute), then each core computes a 512-column
shard of the final wo projection, accumulating gathered head-blocks in
arrival order. Host concatenates the column shards — no all-reduce.

v4 over v2: per-head collectives (wo-tail stall ~100us -> ~0); bf16
broadcast matmuls and a bf16 1/Z (fp32 matmul is 4 cyc/row on the PE);
the softmax Z-path split DVE/PE (3 wide adds reduce 16 expT slices to
4 partials in wbuf0, the PE's ones-matmul contracts the rest); the
first two attention units' QK+exp slices interleaved into the half-1
q/k projection groups so the scalar engine's exp stream (the attention
pacer at ~21.6us/unit) gets a two-unit head start; per-head rope fired
as soon as its q/k groups finish; the z-chain pops placed at sk0/2/8
of the pass1 windows so the in-order DVE queue never blocks it; wo
weight/activation tiles prefetched during attention; 4-slot wv slab
ring; final store split per (dout, s-chunk).

Layouts are all "transposed" ([feature, seq]) so the PE never needs a
transpose: scoresT = kT.T @ qT, exp on ScalarE (PSUM->SBUF, bf16), PV
uses v as the stationary operand (outT = v.T @ expT), the softmax
denominator comes from a ones-column matmul, and the per-position 1/Z
broadcast is a K=1 outer-product matmul whose issue is deferred into
the next unit's matmul stream (keeps the reciprocal off the PE
critical path).

RoPE runs in an even/odd-permuted head basis (host permutes wq/wk
columns; q.k dot products are permutation invariant), which turns the
pair-swap into two 64-partition SBUF->SBUF DMAs plus three elementwise
DVE ops per head.

Raw bass (no Tile): this container's walrus rejects any instruction
with >1 attached sync-wait, so every dependency is an explicit
standalone wait_ge and semaphores are managed manually via cumulative
counters (one counter per buffer family / producer engine).
"""

import sys

sys.path.insert(0, "/opt/trn_rl_repo")

import numpy as np
import ml_dtypes

import concourse.bass as bass
import concourse.mybir as mybir
from concourse.bass_utils import run_bass_kernel_spmd
from concourse import bass_utils as _bu

_orig_run_command = _bu.run_command


def _patched_run_command(cmd, **kw):
    cmd = ["--enable-ldw-opt=true" if c == "--enable-ldw-opt=false" else c
           for c in cmd]
    return _orig_run_command(cmd, **kw)


_bu.run_command = _patched_run_command

N_CORES = 8
S = 2048
D = 4096
NH = 4            # local heads
HD = 128
NL = 512          # local feature columns
NKB = 32          # 128-row blocks over D
SCALE = 1.0 / float(np.sqrt(HD))

BF16 = mybir.dt.bfloat16
F32 = mybir.dt.float32
AF = mybir.ActivationFunctionType
ALU = mybir.AluOpType

ENGINES = ("sync", "tensor", "scalar", "vector", "gpsimd")
LAST_SCHED = None


class Sched:
    """Per-engine straight-line programs with cumulative-counter sems."""

    def __init__(self):
        self.prog = {e: [] for e in ENGINES}
        self.count = {}

    def emit(self, eng, fn, deps=(), inc=None, amt=1):
        dd = {}
        for sem, thr in deps:
            if thr is not None and thr > dd.get(sem, -1):
                dd[sem] = thr
        tick = None
        if inc is not None:
            tick = self.count.get(inc, 0) + amt
            self.count[inc] = tick
        self.prog[eng].append((fn, sorted(dd.items()), inc, amt))
        return tick

    def run(self, eng_name, eng, sems):
        observed = {}
        for fn, deps, inc, amt in self.prog[eng_name]:
            for sem, thr in deps:
                if observed.get(sem, 0) < thr:
                    eng.wait_ge(sems[sem], thr)
                    observed[sem] = thr
            inst = fn(eng)
            if inc is not None:
                inst.then_inc(sems[inc], amt)


def build():
    nc = bass.Bass(num_devices=N_CORES, dynamic_dma_scratch_size=64)

    xT_ext = nc.declare_dram_parameter("xT", [D, S], BF16, isOutput=False)
    wq_ext = nc.declare_dram_parameter("wq", [D, NL], BF16, isOutput=False)
    wk_ext = nc.declare_dram_parameter("wk", [D, NL], BF16, isOutput=False)
    wv_ext = nc.declare_dram_parameter("wv", [D, NL], BF16, isOutput=False)
    wo_ext = nc.declare_dram_parameter("wo", [D, NL], BF16, isOutput=False)
    cosd_ext = nc.declare_dram_parameter("cosd", [HD, S], BF16, isOutput=False)
    sins_ext = nc.declare_dram_parameter("sins", [HD, S], BF16, isOutput=False)
    out_ext = nc.declare_dram_parameter("out", [NL, S], F32, isOutput=True)

    cc_in = nc.dram_tensor("cc_in", [NL, S], BF16)
    cc_out = [
        nc.dram_tensor(f"cc_out{i}", [N_CORES * 128, S], BF16, addr_space="Shared")
        for i in range(NH)
    ]

    sem_names = [
        "xa", "xb", "wb0", "wb1", "wb2", "cs", "swp",
        "ast0", "ast1", "ast2", "ast3",
        "sl0", "sl1", "sl2", "sl3",
        "wl0", "wl1", "wl2", "wl3", "wl4", "wl5", "wl6", "wl7",
        "af0", "af1", "af2", "af3", "af4", "af5", "af6", "af7",
        "yst", "pe", "act", "dve", "cc",
    ]
    # (slab now has 4 slots; sl0..sl3 already declared)

    import contextlib

    with contextlib.ExitStack() as ctx:
        def sb(name, shape, dt):
            return ctx.enter_context(nc.sbuf_tensor(name, shape, dt))

        arenaA = sb("arenaA", [128, 32 * 1024], BF16)   # x half0 -> expT slabs
        arenaB = sb("arenaB", [128, 32 * 1024], BF16)   # x half1 -> afbuf/y_sb
        wbuf = [sb(f"wbuf{i}", [128, NKB * 128], BF16) for i in range(3)]
        slab = sb("slab", [128, 4 * 512], BF16)         # wv stream tiles
        qT_sb = sb("qT_sb", [128, NH * S], BF16)
        kT_sb = sb("kT_sb", [128, NH * S], BF16)
        v_sb = sb("v_sb", [128, 16 * 512], BF16)
        cosd_sb = sb("cosd_sb", [128, S], BF16)
        sins_sb = sb("sins_sb", [128, S], BF16)
        recip_sb = sb("recip", [1, 1024], F32)
        ones_col = sb("onesc", [128, 1], BF16)
        ones_row = sb("onesr", [1, 128], BF16)

        # aliases (temporal reuse, enforced by the schedule):
        qsw = wbuf[0][:, 0:2048]       # rope swap scratch (post q/k groups)
        t1 = wbuf[0][:, 2048:4096]
        t2 = wbuf[1][:, 0:2048]
        # attention-time scratch in arenaB's tail (x half1 dead by then)
        stg = [arenaB[:, 19456 + i * 1024: 19456 + (i + 1) * 1024]
               for i in range(4)]
        recip_bf = arenaB[0:1, 23552:24576]
        # attention normalize scratch aliases rope scratch (dead post-rope;
        # DVE program order separates the uses)
        bc_sb = arenaB[:, 17408:19456].bitcast(F32)  # [128, 1024] f32
        zacc = arenaB[:, 16384:17408]                # [128, 1024] bf16
        woslab = wbuf[2][:, 0:4096]    # 8 x [128,512] wo weight tiles
        afbuf = ([arenaB[:, i * 1024:(i + 1) * 1024] for i in range(4)] +
                 [arenaB[:, 12288 + i * 1024: 12288 + (i + 1) * 1024]
                  for i in range(4)])  # wo rhs, 4 slots per half
        y_sb = arenaB[:, 4096:12288].bitcast(F32)       # [128, 4096] f32

        pairs = [ctx.enter_context(nc.psum_tensor(f"pair{i}", [128, 1024], F32))
                 for i in range(4)]

        sems = {n: ctx.enter_context(nc.semaphore(n)) for n in sem_names}

        sch = Sched()
        global LAST_SCHED
        LAST_SCHED = sch
        _schedule(sch, locals())

        with nc.Block() as block:

            @block.sync
            def _(eng):
                sch.run("sync", eng, sems)

            @block.tensor
            def _(eng):
                sch.run("tensor", eng, sems)

            @block.scalar
            def _(eng):
                sch.run("scalar", eng, sems)

            @block.vector
            def _(eng):
                with nc.allow_low_precision(
                        reason="1/Z kept in bf16 for the broadcast matmul"):
                    sch.run("vector", eng, sems)

            @block.gpsimd
            def _(eng):
                sch.run("gpsimd", eng, sems)

    return nc


def _schedule(sch, env):
    def g(n):
        return env[n]

    xT_ext, wq_ext, wk_ext, wv_ext, wo_ext = (
        g("xT_ext"), g("wq_ext"), g("wk_ext"), g("wv_ext"), g("wo_ext"))
    cosd_ext, sins_ext, out_ext = g("cosd_ext"), g("sins_ext"), g("out_ext")
    cc_in, cc_out = g("cc_in"), g("cc_out")
    arenaA, arenaB, wbuf, slab = g("arenaA"), g("arenaB"), g("wbuf"), g("slab")
    qT_sb, kT_sb, v_sb = g("qT_sb"), g("kT_sb"), g("v_sb")
    cosd_sb, sins_sb = g("cosd_sb"), g("sins_sb")
    qsw, t1, t2 = g("qsw"), g("t1"), g("t2")
    stg, recip_sb, bc_sb = g("stg"), g("recip_sb"), g("bc_sb")
    recip_bf = g("recip_bf")
    zacc = g("zacc")
    woslab = g("woslab")
    ones_col, ones_row = g("ones_col"), g("ones_row")
    afbuf, y_sb = g("afbuf"), g("y_sb")
    pairs = g("pairs")
    arenas = [arenaA, arenaB]

    E = sch.emit

    def dma(out_ap, in_ap):
        return lambda eng: eng.dma_start(out=out_ap, in_=in_ap)

    # ---------------- SP: x loads (both halves up front) ----------------
    xsem = {}

    def emit_x_load(half, j):
        name = "xa" if half == 0 else "xb"
        xsem[half] = E("sync", dma(
            arenas[half][:, j * 8192:(j + 1) * 8192]
            .rearrange("p (kb s) -> p kb s", kb=8),
            xT_ext[j * 1024:(j + 1) * 1024, half * 1024:(half + 1) * 1024]
            .rearrange("(kb p) s -> p kb s", p=128)),
            inc=name, amt=16)

    # wb0 first so group 0 can start as soon as the first x quarter lands
    E("vector", lambda eng: eng.memset(ones_col[:], 1.0), inc="dve")
    dve_ones = E("vector", lambda eng: eng.memset(ones_row[:], 1.0), inc="dve")

    # ---------------- projections ----------------
    bank_war = {}      # (pair_idx, colhalf) -> act tick of last reader
    evac_tick = {}     # ("q"/"k", n, half) -> act tick
    wgrp = {}          # qk group idx -> pe tick of its last matmul
    vk_tick = {}       # (half, k) -> pe tick (for slab WAR)
    v_end = {}
    wb_tick = {}
    sl_count = {i: 0 for i in range(4)}

    GL1 = [0, 4, 1, 5, 2, 6, 3, 7]   # half-1 group order: q0 k0 q1 k1 ...

    def emit_qk_weight_dma(gg):
        half = gg // 8
        gl = GL1[gg - 8] if half == 1 else gg % 8
        t, n = ("q", gl) if gl < 4 else ("k", gl - 4)
        wext = wq_ext if t == "q" else wk_ext
        slot = gg % 3
        deps = []
        if wgrp.get(gg - 3) is not None:
            deps.append(("pe", wgrp[gg - 3]))
        wb_tick[gg] = E("sync", dma(
            wbuf[slot][:].rearrange("p (kb c) -> p kb c", kb=NKB),
            wext[:, n * 128:(n + 1) * 128].rearrange("(kb p) c -> p kb c", p=128)),
            deps=deps, inc=f"wb{slot}", amt=16)

    emit_x_load(0, 0)
    emit_qk_weight_dma(0)
    emit_qk_weight_dma(1)
    emit_x_load(0, 1)
    emit_x_load(0, 2)
    emit_x_load(0, 3)
    E("sync", dma(cosd_sb[:], cosd_ext[:]), inc="cs", amt=16)
    CS_ALL = E("sync", dma(sins_sb[:], sins_ext[:]), inc="cs", amt=16)
    preload_slabs = True

    def emit_one_qk_group(gg, half, gl, pidx):
        t, n = ("q", gl) if gl < 4 else ("k", gl - 4)
        slot = gg % 3
        pair = pairs[pidx]
        xname = "xa" if half == 0 else "xb"
        deps = [(f"wb{slot}", wb_tick[gg])]
        if gg >= 4:
            deps.append((xname, 64))
        else:
            deps.append((xname, 16))
        for chf in range(2):
            if bank_war.get((pidx, chf)) is not None:
                deps.append(("act", bank_war[(pidx, chf)]))
        tick = None
        for k in range(NKB):
            kdeps = ()
            if k == 0:
                kdeps = deps
            elif gg < 4 and k % 8 == 0:
                kdeps = [(xname, 16 * (k // 8 + 1))]
            for sc in range(2):
                last = (k == NKB - 1) and (sc == 1)

                def mm(eng, k=k, sc=sc, pair=pair, slot=slot, half=half):
                    return eng.matmul(
                        pair[:, sc * 512:(sc + 1) * 512],
                        wbuf[slot][:, k * 128:(k + 1) * 128],
                        arenas[half][:, k * 1024 + sc * 512:
                                     k * 1024 + (sc + 1) * 512],
                        start=(k == 0), stop=(k == NKB - 1))

                tick = E("tensor", mm,
                         deps=kdeps if sc == 0 else (),
                         inc="pe" if last else None)
        wgrp[gg] = tick
        dst = qT_sb if t == "q" else kT_sb

        def evac(eng, dst=dst, n=n, half=half, pair=pair):
            return eng.copy(
                dst[:, n * S + half * 1024: n * S + (half + 1) * 1024],
                pair[:, 0:1024])

        a = E("scalar", evac, deps=[("pe", tick)], inc="act")
        bank_war[(pidx, 0)] = a
        bank_war[(pidx, 1)] = a
        evac_tick[(t, n, half)] = a

    def emit_qk_groups0():
        for gl in range(8):
            emit_one_qk_group(gl, 0, gl, gl % 2)
            if gl + 2 <= 7:
                emit_qk_weight_dma(gl + 2)
            if gl in (1, 3, 5, 7):
                emit_x_load(1, (gl - 1) // 2)

    def emit_qk_groups_h1():
        # half-1 q/k groups in q0,k0,q1,k1,... order on psum pairs 2/3,
        # with per-head rope and the first two attention units' QK+exp
        # slices interleaved (their exps hide under the projection PE work)
        for p in range(8):
            gg = 8 + p
            emit_one_qk_group(gg, 1, GL1[p], 2 + p % 2)
            if gg + 3 <= 15:
                emit_qk_weight_dma(gg + 3)
            if p % 2 == 1:
                hh = p // 2
                emit_rope("q", qT_sb, hh)
                emit_rope("k", kT_sb, hh)

    def emit_slab(half, k):
        slot = k % 4
        war = vk_tick.get((half, k - 4))
        if war is None and half == 1:
            war = vk_tick.get((0, k + NKB - 4))
        deps = [("pe", war)] if war is not None else []
        sl_count[slot] += 16
        E("sync", dma(
            slab[:, slot * 512:(slot + 1) * 512],
            wv_ext[k * 128:(k + 1) * 128, :]),
          deps=deps, inc=f"sl{slot}", amt=16)

    def emit_v_groups(half):
        tick = None
        for k in range(NKB):
            slot = k % 4
            deps = [(f"sl{slot}", 16 * (half * 8 + k // 4 + 1))]
            if k == 0:
                for pidx in range(4):
                    for chf in range(2):
                        if bank_war.get((pidx, chf)) is not None:
                            deps.append(("act", bank_war[(pidx, chf)]))
            for st in range(8):
                last = st == 7

                def mmv(eng, k=k, st=st, half=half, slot=slot):
                    return eng.matmul(
                        pairs[st // 2][:, (st % 2) * 512:(st % 2 + 1) * 512],
                        arenas[half][:, k * 1024 + st * 128:
                                     k * 1024 + st * 128 + 128],
                        slab[:, slot * 512:(slot + 1) * 512],
                        start=(k == 0), stop=(k == NKB - 1))

                tick = E("tensor", mmv, deps=deps if st == 0 else (),
                         inc="pe" if last else None)
            vk_tick[(half, k)] = tick
            if k + 4 < NKB:
                emit_slab(half, k + 4)
            if half == 0 and k < 2:
                emit_qk_weight_dma(8 + k)
            if half == 0 and k == 2:
                emit_qk_weight_dma(10)
        v_end[half] = tick
        for st in range(8):
            stg_idx = half * 8 + st

            def evacv(eng, stg_idx=stg_idx, st=st):
                return eng.copy(
                    v_sb[:, stg_idx * 512:(stg_idx + 1) * 512],
                    pairs[st // 2][:, (st % 2) * 512:(st % 2 + 1) * 512])

            evdeps = [("pe", v_end[half])]
            if half == 1:
                evdeps.append(("dve", rope_last))  # v_sb rope-scratch WAR
            a = E("scalar", evacv, deps=evdeps, inc="act")
            bank_war[(st // 2, st % 2)] = a

    # ---------------- RoPE (in-place, v_sb tail scratch) ----------------
    swp = 0
    prev_sw = None
    rope_last = None
    rope_done = {}
    rp_t1 = v_sb[:, 4096:6144]   # v(h1) region: free until v(h1) evacs

    def emit_rope(t, tsb, h):
        nonlocal swp, prev_sw, rope_last
        c0 = h * S
        d0 = [("act", evac_tick[(t, h, 0)]), ("act", evac_tick[(t, h, 1)])]
        dsw = d0 + ([("dve", prev_sw)] if prev_sw is not None else [])
        swp = E("scalar", dma(v_sb[0:64, 6144:8192],
                              tsb[64:128, c0:c0 + S]),
                deps=dsw, inc="swp", amt=16)
        swp = E("scalar", dma(v_sb[64:128, 6144:8192],
                              tsb[0:64, c0:c0 + S]), inc="swp", amt=16)

        def f_t1(eng, tsb=tsb, c0=c0):
            return eng.tensor_mul(rp_t1, tsb[:, c0:c0 + S], cosd_sb[:])

        E("vector", f_t1, deps=d0 + [("cs", CS_ALL)], inc="dve")

        def f_t2(eng, tsb=tsb, c0=c0):
            return eng.tensor_mul(tsb[:, c0:c0 + S], v_sb[:, 6144:8192],
                                  sins_sb[:])

        prev_sw = E("vector", f_t2, deps=[("swp", swp)], inc="dve")

        def f_add(eng, tsb=tsb, c0=c0):
            return eng.tensor_add(tsb[:, c0:c0 + S], tsb[:, c0:c0 + S],
                                  rp_t1)

        rope_last = rope_done[(t, h)] = E("vector", f_add, inc="dve")

    # ------------- attention: 8 half-units (head-major) -------------
    # dunit d = h*2 + qp covers head h, s-half qp (two sq quarters).
    # expT slab (d%2) = arenaA[:, (d%2)*16384 : +16384] as [16 sk][1024].
    # AllGather is per local head (4 gathers): gather(h) fires as soon as
    # units 2h, 2h+1 have stored, so gathers h0-h2 hide under attention.
    # wo kb enumerates (head, core): kb = h*8 + c -> gathered rows
    # cc_out[h][c*128:...], weight rows wo_ext[(c*4 + h)*128:...].
    wo_kb_tick = {}
    af = {i: 0 for i in range(8)}
    wl_count = {i: 0 for i in range(8)}
    af_loaded = set()
    wl_loaded = set()

    def emit_afbuf_load(half, kb, engine="sync"):
        if (half, kb) in af_loaded:
            return
        af_loaded.add((half, kb))
        h, c = kb // 8, kb % 8
        aslot = half * 4 + kb % 4
        war = wo_kb_tick.get((half, kb - 4))
        deps = [("cc", h + 1)]
        if war is not None:
            deps.append(("pe", war))
        af[aslot] += 16
        E(engine, dma(
            afbuf[aslot],
            cc_out[h][c * 128:(c + 1) * 128,
                      half * 1024:(half + 1) * 1024]),
          deps=deps, inc=f"af{aslot}", amt=16)

    def emit_woslab_load(half, kb, engine="sync"):
        if (half, kb) in wl_loaded:
            return
        wl_loaded.add((half, kb))
        h, c = kb // 8, kb % 8
        wslot = kb % 8
        sdeps = [("pe", wgrp[14])]   # wbuf[2] WAR (last qk reader)
        swar = wo_kb_tick.get((half, kb - 8))
        if swar is None and half == 1:
            swar = wo_kb_tick.get((0, kb + NKB - 8))
        if swar is not None:
            sdeps.append(("pe", swar))
        wl_count[wslot] += 16
        wrow = (c * NH + h) * 128
        E(engine, dma(
            woslab[:, wslot * 512:(wslot + 1) * 512],
            wo_ext[wrow:wrow + 128, :]),
          deps=sdeps, inc=f"wl{wslot}", amt=16)

    def emit_wo_loads(half, kb, engine="sync"):
        emit_afbuf_load(half, kb, engine)
        emit_woslab_load(half, kb, engine)

    exp_last = {}
    pv_read_end = {}
    state = {"stt": None, "recip": None, "bc": None, "bcast": None,
             "zmm": None, "adds": {}}
    stg_store = {}
    store_tick = {}
    ps_o_pair = {}
    ast = {0: 0, 1: 0, 2: 0, 3: 0}
    pending_zr = []
    pending_bc = []
    pending_adds = []
    adds_l1 = {}

    def finish_unit(d, bcast_tick):
        h, qp = d // 2, d % 2
        state["bc"] = E(
            "vector",
            lambda eng: eng.tensor_copy(bc_sb[:], pairs[3][:, 0:1024]),
            deps=[("pe", bcast_tick)], inc="dve")
        slot = d % 4
        sdeps = []
        if slot in stg_store:
            sdeps.append(stg_store[slot])

        def f_stt(eng, slot=slot, d=d):
            return eng.scalar_tensor_tensor(
                stg[slot][:], pairs[ps_o_pair[d]][:, 0:1024], 1.0, bc_sb[:],
                ALU.mult, ALU.mult)

        state["stt"] = E("vector", f_stt, deps=sdeps, inc="dve")

        sem = f"ast{slot}"
        ast[slot] += 16
        E("sync", dma(
            cc_in[h * 128:(h + 1) * 128, qp * 1024:(qp + 1) * 1024],
            stg[slot][:]),
            deps=[("dve", state["stt"])], inc=sem, amt=16)
        stg_store[slot] = (sem, ast[slot])
        store_tick[d] = (sem, ast[slot])

    def make_zr(d):
        def emit_zr():
            dps = [("dve", state["adds"][d]), ("dve", dve_ones)]
            if state["recip"] is not None:
                dps.append(("dve", state["recip"]))  # ps_z WAR
            if state["bc"] is not None:
                dps.append(("dve", state["bc"]))  # pairs[3] WAR vs bc copy
            for chf in range(2):
                bw = bank_war.get((3, chf))
                if bw is not None:
                    dps.append(("act", bw))  # pairs[3] WAR vs v(h1) evacs
            # Z = ones.T @ partials: contract the remaining 4 partial
            # slices on the PE (8 accumulating mms) instead of more DVE adds
            zmm = None
            for ch in range(4):
                for zc in range(2):
                    def fz(eng, ch=ch, zc=zc):
                        return eng.matmul(
                            pairs[3][0:1, zc * 512:(zc + 1) * 512], ones_col[:],
                            wbuf[0][:, ch * 1024 + zc * 512:
                                    ch * 1024 + (zc + 1) * 512],
                            start=(ch == 0), stop=(ch == 3))
                    last = ch == 3 and zc == 1
                    zmm = E("tensor", fz,
                            deps=dps if (ch == 0 and zc == 0) else (),
                            inc="pe" if last else None)
            state["zmm"] = zmm
            rdeps = [("pe", zmm)]
            if state["bcast"] is not None:
                rdeps.append(("pe", state["bcast"]))  # recip_bf WAR
            state["recip"] = E(
                "vector",
                lambda eng: eng.reciprocal(recip_bf[:], pairs[3][0:1, 0:1024]),
                deps=rdeps, inc="dve")
        return emit_zr

    def make_bcast(d):
        def emit_bcast():
            dps = [("dve", state["recip"])]
            if state["bc"] is not None:
                dps.append(("dve", state["bc"]))
            for chf in range(2):
                bw = bank_war.get((3, chf))
                if bw is not None:
                    dps.append(("act", bw))  # pairs[3] WAR vs v(h1) evacs
            bt = None
            for zc in range(2):
                def fb(eng, zc=zc):
                    return eng.matmul(
                        pairs[3][:, zc * 512:(zc + 1) * 512], ones_row[:],
                        recip_bf[:, zc * 512:(zc + 1) * 512],
                        start=True, stop=True)
                bt = E("tensor", fb, deps=dps if zc == 0 else (),
                       inc="pe" if zc == 1 else None)
            state["bcast"] = bt
            finish_unit(d, bt)
        return emit_bcast

    def emit_pass1(d, sks=None):
        h, qp = d // 2, d % 2
        base = (d % 2) * 16384
        if sks is None:
            sks = range(16)
        for sk in sks:
            pidx = sk % 2
            pair = pairs[pidx]
            deps = [("dve", rope_done[("q", h)]), ("dve", rope_done[("k", h)])]
            for chf in range(2):
                if bank_war.get((pidx, chf)) is not None:
                    deps.append(("act", bank_war[(pidx, chf)]))
            tick = None
            for qi in range(2):

                def mm1(eng, pair=pair, h=h, sk=sk, qp=qp, qi=qi):
                    return eng.matmul(
                        pair[:, qi * 512:(qi + 1) * 512],
                        kT_sb[:, h * S + sk * 128: h * S + sk * 128 + 128],
                        qT_sb[:, h * S + qp * 1024 + qi * 512:
                              h * S + qp * 1024 + (qi + 1) * 512],
                        start=True, stop=True)

                tick = E("tensor", mm1, deps=deps if qi == 0 else (),
                         inc="pe" if qi == 1 else None)

            if sk == 0 and pending_zr:
                pending_zr.pop(0)()
            if sk == 2 and pending_adds:
                pending_adds.pop(0)()
            if sk == 8 and pending_bc:
                pending_bc.pop(0)()

            edeps = [("pe", tick)]
            if d >= 2 and sk == 0:
                edeps.append(("pe", pv_read_end[d - 2]))
                edeps.append(("dve", adds_l1[d - 2]))
            if d < 2 and sk == 0:
                edeps.append(("pe", P_H0_END))  # arenaA WAR vs half0 x

            def f_exp(eng, pair=pair, base=base, sk=sk):
                return eng.activation(
                    arenaA[:, base + sk * 1024: base + (sk + 1) * 1024],
                    pair[:, 0:1024], AF.Exp, scale=SCALE)

            a = E("scalar", f_exp, deps=edeps, inc="act")
            exp_last[d] = a
            bank_war[(pidx, 0)] = a
            bank_war[(pidx, 1)] = a
        if 15 in sks:
            pending_adds.append(make_adds(d))

    def emit_pass2(d, opair=2):
        h, qp = d // 2, d % 2
        base = (d % 2) * 16384
        deps = [("act", exp_last[d])]
        if opair == 2 and state["stt"] is not None:
            deps.append(("dve", state["stt"]))
        # pair WAR + v_sb RAW vs the v(h1) evacs: units 0/1's exps precede
        # the v evacs on the act queue, so program order no longer covers
        # it. (3,1) is the last v evac; stale-but-harmless for later units.
        bw = bank_war.get((3, 1))
        if bw is not None:
            deps.append(("act", bw))
        for chf in range(2):
            bw = bank_war.get((opair, chf))
            if bw is not None:
                deps.append(("act", bw))
        tick = None
        for sk in range(16):
            for qi in range(2):

                def mpv(eng, sk=sk, h=h, base=base, qi=qi, opair=opair):
                    return eng.matmul(
                        pairs[opair][:, qi * 512:(qi + 1) * 512],
                        v_sb[:, sk * 512 + h * 128: sk * 512 + h * 128 + 128],
                        arenaA[:, base + sk * 1024 + qi * 512:
                               base + sk * 1024 + (qi + 1) * 512],
                        start=(sk == 0), stop=(sk == 15))

                tick = E("tensor", mpv,
                         deps=deps if (sk == 0 and qi == 0) else (),
                         inc="pe" if (sk == 15 and qi == 1) else None)
        pv_read_end[d] = tick
        ps_o_pair[d] = opair

        pending_zr.append(make_zr(d))
        pending_bc.append(make_bcast(d))

    def make_adds(d):
        # reduce 16 expT slices to 4 partials in wbuf[0] (dead after the
        # last qk group; the PE's zmm contracts the rest). Popped into the
        # NEXT pass1 window so the DVE queue never blocks the z-chain.
        base = (d % 2) * 16384

        def emit_adds():
            adeps = [("act", exp_last[d]), ("pe", wgrp[15])]
            if state["zmm"] is not None:
                adeps.append(("pe", state["zmm"]))  # scratch WAR vs zmm reads

            def fa(eng, q, acc, base=base):
                sl = arenaA[:, base + q * 4096:base + (q + 1) * 4096]
                if not acc:
                    return eng.tensor_add(
                        wbuf[0][:, 0:4096],
                        sl, arenaA[:, base + 4096 * (q + 1):
                                   base + 4096 * (q + 2)])
                return eng.tensor_add(wbuf[0][:, 0:4096],
                                      wbuf[0][:, 0:4096], sl)

            E("vector", lambda eng: fa(eng, 0, False), deps=adeps)
            E("vector", lambda eng: fa(eng, 2, True))
            tick = E("vector", lambda eng: fa(eng, 3, True), inc="dve")
            adds_l1[d] = state["adds"][d] = tick
        return emit_adds

    # PE order: qk(h0), v(h0), qk(h1)+rope+pass1(0,1), v(h1), attention
    for k in range(4):
        emit_slab(0, k)
    emit_qk_groups0()
    emit_v_groups(0)
    P_H0_END = v_end[0]
    for k in range(4):
        emit_slab(1, k)
    emit_qk_groups_h1()
    emit_v_groups(1)

    emit_pass1(0)
    emit_pass1(1)
    for d in range(2, 8):
        emit_pass2(d - 2)
        emit_pass1(d)
        if d == 3:
            # woslab tiles for head 0 prefetch during early attention
            # (wbuf[2] WAR only — no collective dependency)
            for kb_pre in range(8):
                emit_woslab_load(0, kb_pre)
    # afbuf preloads sit on the sync queue BEFORE the last units' cc_in
    # stores; their cc(1) dep is satisfied mid-attention so they stream in
    # well before the wo matmuls need them
    for kb_pre in range(4):
        emit_afbuf_load(0, kb_pre)
    emit_pass2(6)             # PV(6); queues zr6/bc6
    pending_zr.pop(0)()       # zmm(6)+recip(6): reads ztree(6) before L1(7)
    pending_adds.pop(0)()     # tree(7), gated on zmm(6) via ztree WAR
    emit_pass2(7, opair=0)    # PV(7) -> pair0, overlaps unit 6's chain
    pending_bc.pop(0)()       # bcast(6)+stt(6)
    pending_zr.pop(0)()       # zmm(7)+recip(7)
    pending_bc.pop(0)()       # bcast(7)+stt(7)

    for h in range(NH):

        def f_ag(eng, h=h):
            return eng.collective_compute(
                "AllGather", ALU.bypass,
                replica_groups=[list(range(N_CORES))],
                ins=[cc_in[h * 128:(h + 1) * 128, :].opt()],
                outs=[cc_out[h][:].opt()])

        E("gpsimd", f_ag,
          deps=[store_tick[2 * h], store_tick[2 * h + 1]],
          inc="cc")

    # ---------------- wo projection ----------------
    y_evac_by_dout = {}
    for half in range(2):
        for kb in range(NKB):
            slot = kb % 4
            aslot = half * 4 + slot
            wslot = kb % 8
            emit_wo_loads(half, kb)

            mmdeps = [(f"af{aslot}", af[aslot]), (f"wl{wslot}", wl_count[wslot])]
            if kb == 0 and half == 0:
                mmdeps.append(("act", exp_last[7]))
                mmdeps.append(("dve", state["stt"]))
                mmdeps.append(("dve", state["recip"]))
            tick = None
            for dout in range(4):
                for c2 in range(2):
                    dd = mmdeps if (dout == 0 and c2 == 0) else []
                    if kb == 0 and half == 1 and c2 == 0:
                        # pairs[dout] WAR: only needs half-0's evacs of
                        # this dout, not the full evac+store tail
                        dd = list(dd) + [("act", y_evac_by_dout[dout])]

                    def mmo(eng, kb=kb, dout=dout, c2=c2,
                            aslot=aslot, wslot=wslot):
                        return eng.matmul(
                            pairs[dout][:, c2 * 512:(c2 + 1) * 512],
                            woslab[:, wslot * 512 + dout * 128:
                                   wslot * 512 + dout * 128 + 128],
                            afbuf[aslot][:, c2 * 512:(c2 + 1) * 512],
                            start=(kb == 0), stop=(kb == NKB - 1))

                    tick = E("tensor", mmo,
                             deps=dd,
                             inc="pe" if (dout == 3 and c2 == 1) else None)
            wo_kb_tick[(half, kb)] = tick

        wo_end = wo_kb_tick[(half, NKB - 1)]
        if half == 0:
            for kb_pre in range(4):
                emit_wo_loads(1, kb_pre, engine="scalar")
        for c2 in range(2):
            for dout in range(4):
                deps = [("pe", wo_end)]
                if half == 1:
                    deps.append(("yst", 16 * (c2 * 4 + dout + 1)))

                def evy(eng, dout=dout, c2=c2):
                    return eng.copy(
                        y_sb[:, (dout * 2 + c2) * 512:(dout * 2 + c2 + 1) * 512],
                        pairs[dout][:, c2 * 512:(c2 + 1) * 512])

                y_evac_last = E("scalar", evy, deps=deps, inc="act")
                if c2 == 1:
                    y_evac_by_dout[dout] = y_evac_last

                cbase = half * 1024 + c2 * 512
                E("sync", dma(
                    out_ext[dout * 128:(dout + 1) * 128, cbase:cbase + 512],
                    y_sb[:, (dout * 2 + c2) * 512:(dout * 2 + c2 + 1) * 512]),
                    deps=[("act", y_evac_last)], inc="yst", amt=16)

    E("sync", lambda eng: None, deps=[("yst", 256)])


# ======================= host side =======================

_NC_CACHE = None


def _get_nc():
    global _NC_CACHE
    if _NC_CACHE is None:
        _NC_CACHE = build()
    return _NC_CACHE


def _prep_inputs(x, freqs_cos, freqs_sin, wq, wk, wv, wo):
    bf = ml_dtypes.bfloat16
    x2 = np.asarray(x, np.float32).reshape(S, D)
    xT = np.ascontiguousarray(x2.T).astype(bf)
    perm = np.concatenate([np.arange(0, HD, 2), np.arange(1, HD, 2)])
    cos = np.asarray(freqs_cos, np.float32)
    sin = np.asarray(freqs_sin, np.float32)
    cosd = np.concatenate([cos.T, cos.T], axis=0).astype(bf)
    sins = np.concatenate([-sin.T, sin.T], axis=0).astype(bf)

    in_maps = []
    for c in range(N_CORES):
        cols_qk = np.concatenate([c * NL + h * HD + perm for h in range(NH)])
        cols_n = np.arange(c * NL, (c + 1) * NL)
        in_maps.append({
            "xT": xT,
            "wq": np.ascontiguousarray(np.asarray(wq, np.float32)[:, cols_qk]).astype(bf),
            "wk": np.ascontiguousarray(np.asarray(wk, np.float32)[:, cols_qk]).astype(bf),
            "wv": np.ascontiguousarray(np.asarray(wv, np.float32)[:, cols_n]).astype(bf),
            "wo": np.ascontiguousarray(np.asarray(wo, np.float32)[:, cols_n]).astype(bf),
            "cosd": cosd,
            "sins": sins,
        })
    return in_maps


def run(inputs, trace=False, **kw):
    nc = _get_nc()
    in_maps = _prep_inputs(**inputs)
    res = run_bass_kernel_spmd(nc, in_maps, core_ids=list(range(N_CORES)),
                               trace=trace, **kw)
    yT = np.concatenate([np.asarray(res.results[c]["out"], np.float32)
                         for c in range(N_CORES)], axis=0)
    out = np.ascontiguousarray(yT.T).reshape(1, S, D).astype(np.float32)
    return out, res


def kernel(x, freqs_cos, freqs_sin, wq, wk, wv, wo):
    out, _ = run(dict(x=x, freqs_cos=freqs_cos, freqs_sin=freqs_sin,
                      wq=wq, wk=wk, wv=wv, wo=wo))
    return out



# revision 75
# speedup vs baseline: 1.0454x; 1.0297x over previous
"""Distributed RoPE-attention kernel for 8 TRN2 NeuronCores (v3).

Sharding: tensor-parallel over heads (4 heads/core) for QKV+attention;
the attention output (bf16) is AllGather'd per local head (4 gathers,
fired as each head's two s-half units store, so heads 0-2 gather under
the remaining attention compute), then each core computes a 512-column
shard of the final wo projection, accumulating gathered head-blocks in
arrival order. Host concatenates the column shards — no all-reduce.

v4 over v2: per-head collectives (wo-tail stall ~100us -> ~0); bf16
broadcast matmuls and a bf16 1/Z (fp32 matmul is 4 cyc/row on the PE);
the softmax Z-path split DVE/PE (3 wide adds reduce 16 expT slices to
4 partials in wbuf0, the PE's ones-matmul contracts the rest); the
first two attention units' QK+exp slices interleaved into the half-1
q/k projection groups so the scalar engine's exp stream (the attention
pacer at ~21.6us/unit) gets a two-unit head start; per-head rope fired
as soon as its q/k groups finish; the z-chain pops placed at sk0/2/8
of the pass1 windows so the in-order DVE queue never blocks it; wo
weight/activation tiles prefetched during attention; 4-slot wv slab
ring; final store split per (dout, s-chunk).

Layouts are all "transposed" ([feature, seq]) so the PE never needs a
transpose: scoresT = kT.T @ qT, exp on ScalarE (PSUM->SBUF, bf16), PV
uses v as the stationary operand (outT = v.T @ expT), the softmax
denominator comes from a ones-column matmul, and the per-position 1/Z
broadcast is a K=1 outer-product matmul whose issue is deferred into
the next unit's matmul stream (keeps the reciprocal off the PE
critical path).

RoPE runs in an even/odd-permuted head basis (host permutes wq/wk
columns; q.k dot products are permutation invariant), which turns the
pair-swap into two 64-partition SBUF->SBUF DMAs plus three elementwise
DVE ops per head.

Raw bass (no Tile): this container's walrus rejects any instruction
with >1 attached sync-wait, so every dependency is an explicit
standalone wait_ge and semaphores are managed manually via cumulative
counters (one counter per buffer family / producer engine).
"""

import sys

sys.path.insert(0, "/opt/trn_rl_repo")

import numpy as np
import ml_dtypes

import concourse.bass as bass
import concourse.mybir as mybir
from concourse.bass_utils import run_bass_kernel_spmd
from concourse import bass_utils as _bu

_orig_run_command = _bu.run_command


def _patched_run_command(cmd, **kw):
    cmd = ["--enable-ldw-opt=true" if c == "--enable-ldw-opt=false" else c
           for c in cmd]
    return _orig_run_command(cmd, **kw)


_bu.run_command = _patched_run_command

N_CORES = 8
S = 2048
D = 4096
NH = 4            # local heads
HD = 128
NL = 512          # local feature columns
NKB = 32          # 128-row blocks over D
SCALE = 1.0 / float(np.sqrt(HD))

BF16 = mybir.dt.bfloat16
F32 = mybir.dt.float32
AF = mybir.ActivationFunctionType
ALU = mybir.AluOpType

ENGINES = ("sync", "tensor", "scalar", "vector", "gpsimd")
LAST_SCHED = None


class Sched:
    """Per-engine straight-line programs with cumulative-counter sems."""

    def __init__(self):
        self.prog = {e: [] for e in ENGINES}
        self.count = {}

    def emit(self, eng, fn, deps=(), inc=None, amt=1):
        dd = {}
        for sem, thr in deps:
            if thr is not None and thr > dd.get(sem, -1):
                dd[sem] = thr
        tick = None
        if inc is not None:
            tick = self.count.get(inc, 0) + amt
            self.count[inc] = tick
        self.prog[eng].append((fn, sorted(dd.items()), inc, amt))
        return tick

    def run(self, eng_name, eng, sems):
        observed = {}
        for fn, deps, inc, amt in self.prog[eng_name]:
            for sem, thr in deps:
                if observed.get(sem, 0) < thr:
                    eng.wait_ge(sems[sem], thr)
                    observed[sem] = thr
            inst = fn(eng)
            if inc is not None:
                inst.then_inc(sems[inc], amt)


def build():
    nc = bass.Bass(num_devices=N_CORES, dynamic_dma_scratch_size=64)

    xT_ext = nc.declare_dram_parameter("xT", [D, S], BF16, isOutput=False)
    wq_ext = nc.declare_dram_parameter("wq", [D, NL], BF16, isOutput=False)
    wk_ext = nc.declare_dram_parameter("wk", [D, NL], BF16, isOutput=False)
    wv_ext = nc.declare_dram_parameter("wv", [D, NL], BF16, isOutput=False)
    wo_ext = nc.declare_dram_parameter("wo", [D, NL], BF16, isOutput=False)
    cosd_ext = nc.declare_dram_parameter("cosd", [HD, S], BF16, isOutput=False)
    sins_ext = nc.declare_dram_parameter("sins", [HD, S], BF16, isOutput=False)
    out_ext = nc.declare_dram_parameter("out", [NL, S], F32, isOutput=True)

    cc_in = nc.dram_tensor("cc_in", [NL, S], BF16)
    cc_out = [
        nc.dram_tensor(f"cc_out{i}", [N_CORES * 128, S], BF16, addr_space="Shared")
        for i in range(NH)
    ]

    sem_names = [
        "xa", "xb", "wb0", "wb1", "wb2", "cs", "swp",
        "ast0", "ast1", "ast2", "ast3",
        "sl0", "sl1", "sl2", "sl3",
        "wl0", "wl1", "wl2", "wl3", "wl4", "wl5", "wl6", "wl7",
        "af0", "af1", "af2", "af3", "af4", "af5", "af6", "af7",
        "yst", "pe", "act", "dve", "cc",
    ]
    # (slab now has 4 slots; sl0..sl3 already declared)

    import contextlib

    with contextlib.ExitStack() as ctx:
        def sb(name, shape, dt):
            return ctx.enter_context(nc.sbuf_tensor(name, shape, dt))

        arenaA = sb("arenaA", [128, 32 * 1024], BF16)   # x half0 -> expT slabs
        arenaB = sb("arenaB", [128, 32 * 1024], BF16)   # x half1 -> afbuf/y_sb
        wbuf = [sb(f"wbuf{i}", [128, NKB * 128], BF16) for i in range(3)]
        slab = sb("slab", [128, 4 * 512], BF16)         # wv stream tiles
        qT_sb = sb("qT_sb", [128, NH * S], BF16)
        kT_sb = sb("kT_sb", [128, NH * S], BF16)
        v_sb = sb("v_sb", [128, 16 * 512], BF16)
        cosd_sb = sb("cosd_sb", [128, S], BF16)
        sins_sb = sb("sins_sb", [128, S], BF16)
        recip_sb = sb("recip", [1, 1024], F32)
        ones_col = sb("onesc", [128, 1], BF16)
        ones_row = sb("onesr", [1, 128], BF16)

        # aliases (temporal reuse, enforced by the schedule):
        qsw = wbuf[0][:, 0:2048]       # rope swap scratch (post q/k groups)
        t1 = wbuf[0][:, 2048:4096]
        t2 = wbuf[1][:, 0:2048]
        # attention-time scratch in arenaB's tail (x half1 dead by then)
        stg = [arenaB[:, 19456 + i * 1024: 19456 + (i + 1) * 1024]
               for i in range(4)]
        recip_bf = arenaB[0:1, 23552:24576]
        # attention normalize scratch aliases rope scratch (dead post-rope;
        # DVE program order separates the uses)
        bc_sb = arenaB[:, 17408:19456].bitcast(F32)  # [128, 1024] f32
        zacc = arenaB[:, 16384:17408]                # [128, 1024] bf16
        woslab = wbuf[2][:, 0:4096]    # 8 x [128,512] wo weight tiles
        afbuf = ([arenaB[:, i * 1024:(i + 1) * 1024] for i in range(4)] +
                 [arenaB[:, 12288 + i * 1024: 12288 + (i + 1) * 1024]
                  for i in range(4)])  # wo rhs, 4 slots per half
        y_sb = arenaB[:, 4096:12288].bitcast(F32)       # [128, 4096] f32

        pairs = [ctx.enter_context(nc.psum_tensor(f"pair{i}", [128, 1024], F32))
                 for i in range(4)]

        sems = {n: ctx.enter_context(nc.semaphore(n)) for n in sem_names}

        sch = Sched()
        global LAST_SCHED
        LAST_SCHED = sch
        _schedule(sch, locals())

        with nc.Block() as block:

            @block.sync
            def _(eng):
                sch.run("sync", eng, sems)

            @block.tensor
            def _(eng):
                sch.run("tensor", eng, sems)

            @block.scalar
            def _(eng):
                sch.run("scalar", eng, sems)

            @block.vector
            def _(eng):
                with nc.allow_low_precision(
                        reason="1/Z kept in bf16 for the broadcast matmul"):
                    sch.run("vector", eng, sems)

            @block.gpsimd
            def _(eng):
                sch.run("gpsimd", eng, sems)

    return nc


def _schedule(sch, env):
    def g(n):
        return env[n]

    xT_ext, wq_ext, wk_ext, wv_ext, wo_ext = (
        g("xT_ext"), g("wq_ext"), g("wk_ext"), g("wv_ext"), g("wo_ext"))
    cosd_ext, sins_ext, out_ext = g("cosd_ext"), g("sins_ext"), g("out_ext")
    cc_in, cc_out = g("cc_in"), g("cc_out")
    arenaA, arenaB, wbuf, slab = g("arenaA"), g("arenaB"), g("wbuf"), g("slab")
    qT_sb, kT_sb, v_sb = g("qT_sb"), g("kT_sb"), g("v_sb")
    cosd_sb, sins_sb = g("cosd_sb"), g("sins_sb")
    qsw, t1, t2 = g("qsw"), g("t1"), g("t2")
    stg, recip_sb, bc_sb = g("stg"), g("recip_sb"), g("bc_sb")
    recip_bf = g("recip_bf")
    zacc = g("zacc")
    woslab = g("woslab")
    ones_col, ones_row = g("ones_col"), g("ones_row")
    afbuf, y_sb = g("afbuf"), g("y_sb")
    pairs = g("pairs")
    arenas = [arenaA, arenaB]

    E = sch.emit

    def dma(out_ap, in_ap):
        return lambda eng: eng.dma_start(out=out_ap, in_=in_ap)

    # ---------------- SP: x loads (both halves up front) ----------------
    xsem = {}

    def emit_x_load(half, j):
        name = "xa" if half == 0 else "xb"
        xsem[half] = E("sync", dma(
            arenas[half][:, j * 8192:(j + 1) * 8192]
            .rearrange("p (kb s) -> p kb s", kb=8),
            xT_ext[j * 1024:(j + 1) * 1024, half * 1024:(half + 1) * 1024]
            .rearrange("(kb p) s -> p kb s", p=128)),
            inc=name, amt=16)

    # wb0 first so group 0 can start as soon as the first x quarter lands
    E("vector", lambda eng: eng.memset(ones_col[:], 1.0), inc="dve")
    dve_ones = E("vector", lambda eng: eng.memset(ones_row[:], 1.0), inc="dve")

    # ---------------- projections ----------------
    bank_war = {}      # (pair_idx, colhalf) -> act tick of last reader
    evac_tick = {}     # ("q"/"k", n, half) -> act tick
    wgrp = {}          # qk group idx -> pe tick of its last matmul
    vk_tick = {}       # (half, k) -> pe tick (for slab WAR)
    v_end = {}
    wb_tick = {}
    sl_count = {i: 0 for i in range(4)}

    GL1 = [0, 4, 1, 5, 2, 6, 3, 7]   # half-1 group order: q0 k0 q1 k1 ...

    def emit_qk_weight_dma(gg):
        half = gg // 8
        gl = GL1[gg - 8] if half == 1 else gg % 8
        t, n = ("q", gl) if gl < 4 else ("k", gl - 4)
        wext = wq_ext if t == "q" else wk_ext
        slot = gg % 3
        deps = []
        if wgrp.get(gg - 3) is not None:
            deps.append(("pe", wgrp[gg - 3]))
        wb_tick[gg] = E("sync", dma(
            wbuf[slot][:].rearrange("p (kb c) -> p kb c", kb=NKB),
            wext[:, n * 128:(n + 1) * 128].rearrange("(kb p) c -> p kb c", p=128)),
            deps=deps, inc=f"wb{slot}", amt=16)

    emit_x_load(0, 0)
    emit_qk_weight_dma(0)
    emit_qk_weight_dma(1)
    emit_x_load(0, 1)
    emit_x_load(0, 2)
    emit_x_load(0, 3)
    E("sync", dma(cosd_sb[:], cosd_ext[:]), inc="cs", amt=16)
    CS_ALL = E("sync", dma(sins_sb[:], sins_ext[:]), inc="cs", amt=16)
    preload_slabs = True

    def emit_one_qk_group(gg, half, gl, pidx):
        t, n = ("q", gl) if gl < 4 else ("k", gl - 4)
        slot = gg % 3
        pair = pairs[pidx]
        xname = "xa" if half == 0 else "xb"
        deps = [(f"wb{slot}", wb_tick[gg])]
        if gg >= 4:
            deps.append((xname, 64))
        else:
            deps.append((xname, 16))
        for chf in range(2):
            if bank_war.get((pidx, chf)) is not None:
                deps.append(("act", bank_war[(pidx, chf)]))
        tick = None
        for k in range(NKB):
            kdeps = ()
            if k == 0:
                kdeps = deps
            elif gg < 4 and k % 8 == 0:
                kdeps = [(xname, 16 * (k // 8 + 1))]
            for sc in range(2):
                last = (k == NKB - 1) and (sc == 1)

                def mm(eng, k=k, sc=sc, pair=pair, slot=slot, half=half):
                    return eng.matmul(
                        pair[:, sc * 512:(sc + 1) * 512],
                        wbuf[slot][:, k * 128:(k + 1) * 128],
                        arenas[half][:, k * 1024 + sc * 512:
                                     k * 1024 + (sc + 1) * 512],
                        start=(k == 0), stop=(k == NKB - 1))

                tick = E("tensor", mm,
                         deps=kdeps if sc == 0 else (),
                         inc="pe" if last else None)
        wgrp[gg] = tick
        dst = qT_sb if t == "q" else kT_sb

        def evac(eng, dst=dst, n=n, half=half, pair=pair):
            return eng.copy(
                dst[:, n * S + half * 1024: n * S + (half + 1) * 1024],
                pair[:, 0:1024])

        a = E("scalar", evac, deps=[("pe", tick)], inc="act")
        bank_war[(pidx, 0)] = a
        bank_war[(pidx, 1)] = a
        evac_tick[(t, n, half)] = a

    def emit_qk_groups0():
        for gl in range(8):
            emit_one_qk_group(gl, 0, gl, gl % 2)
            if gl + 2 <= 7:
                emit_qk_weight_dma(gl + 2)
            if gl in (1, 3, 5, 7):
                emit_x_load(1, (gl - 1) // 2)

    def emit_qk_groups_h1():
        # half-1 q/k groups in q0,k0,q1,k1,... order on psum pairs 2/3,
        # with per-head rope and the first two attention units' QK+exp
        # slices interleaved (their exps hide under the projection PE work)
        for p in range(8):
            gg = 8 + p
            emit_one_qk_group(gg, 1, GL1[p], 2 + p % 2)
            if gg + 3 <= 15:
                emit_qk_weight_dma(gg + 3)
            pass

    def emit_slab(half, k):
        slot = k % 4
        war = vk_tick.get((half, k - 4))
        if war is None and half == 1:
            war = vk_tick.get((0, k + NKB - 4))
        deps = [("pe", war)] if war is not None else []
        sl_count[slot] += 16
        E("sync", dma(
            slab[:, slot * 512:(slot + 1) * 512],
            wv_ext[k * 128:(k + 1) * 128, :]),
          deps=deps, inc=f"sl{slot}", amt=16)

    def emit_v_groups(half):
        tick = None
        for k in range(NKB):
            slot = k % 4
            deps = [(f"sl{slot}", 16 * (half * 8 + k // 4 + 1))]
            if k == 0:
                for pidx in range(4):
                    for chf in range(2):
                        if bank_war.get((pidx, chf)) is not None:
                            deps.append(("act", bank_war[(pidx, chf)]))
            for st in range(8):
                last = st == 7

                def mmv(eng, k=k, st=st, half=half, slot=slot):
                    return eng.matmul(
                        pairs[st // 2][:, (st % 2) * 512:(st % 2 + 1) * 512],
                        arenas[half][:, k * 1024 + st * 128:
                                     k * 1024 + st * 128 + 128],
                        slab[:, slot * 512:(slot + 1) * 512],
                        start=(k == 0), stop=(k == NKB - 1))

                tick = E("tensor", mmv, deps=deps if st == 0 else (),
                         inc="pe" if last else None)
            vk_tick[(half, k)] = tick
            if k + 4 < NKB:
                emit_slab(half, k + 4)
            if half == 0 and k < 2:
                emit_qk_weight_dma(8 + k)
            if half == 0 and k == 2:
                emit_qk_weight_dma(10)
        v_end[half] = tick
        for st in range(8):
            stg_idx = half * 8 + st

            def evacv(eng, stg_idx=stg_idx, st=st):
                return eng.copy(
                    v_sb[:, stg_idx * 512:(stg_idx + 1) * 512],
                    pairs[st // 2][:, (st % 2) * 512:(st % 2 + 1) * 512])

            evdeps = [("pe", v_end[half])]
            if half == 1:
                evdeps.append(("dve", rope_last))  # v_sb rope-scratch WAR
            a = E("scalar", evacv, deps=evdeps, inc="act")
            bank_war[(st // 2, st % 2)] = a

    # ---------------- RoPE (in-place, v_sb tail scratch) ----------------
    swp = 0
    prev_sw = None
    rope_last = None
    rope_done = {}
    rp_t1 = v_sb[:, 4096:6144]   # v(h1) region: free until v(h1) evacs

    def emit_rope(t, tsb, h):
        nonlocal swp, prev_sw, rope_last
        c0 = h * S
        d0 = [("act", evac_tick[(t, h, 0)]), ("act", evac_tick[(t, h, 1)])]
        dsw = d0 + ([("dve", prev_sw)] if prev_sw is not None else [])
        swp = E("scalar", dma(v_sb[0:64, 6144:8192],
                              tsb[64:128, c0:c0 + S]),
                deps=dsw, inc="swp", amt=16)
        swp = E("scalar", dma(v_sb[64:128, 6144:8192],
                              tsb[0:64, c0:c0 + S]), inc="swp", amt=16)

        def f_t1(eng, tsb=tsb, c0=c0):
            return eng.tensor_mul(rp_t1, tsb[:, c0:c0 + S], cosd_sb[:])

        E("vector", f_t1, deps=d0 + [("cs", CS_ALL)], inc="dve")

        def f_t2(eng, tsb=tsb, c0=c0):
            return eng.tensor_mul(tsb[:, c0:c0 + S], v_sb[:, 6144:8192],
                                  sins_sb[:])

        prev_sw = E("vector", f_t2, deps=[("swp", swp)], inc="dve")

        def f_add(eng, tsb=tsb, c0=c0):
            return eng.tensor_add(tsb[:, c0:c0 + S], tsb[:, c0:c0 + S],
                                  rp_t1)

        rope_last = rope_done[(t, h)] = E("vector", f_add, inc="dve")

    # ------------- attention: 8 half-units (head-major) -------------
    # dunit d = h*2 + qp covers head h, s-half qp (two sq quarters).
    # expT slab (d%2) = arenaA[:, (d%2)*16384 : +16384] as [16 sk][1024].
    # AllGather is per local head (4 gathers): gather(h) fires as soon as
    # units 2h, 2h+1 have stored, so gathers h0-h2 hide under attention.
    # wo kb enumerates (head, core): kb = h*8 + c -> gathered rows
    # cc_out[h][c*128:...], weight rows wo_ext[(c*4 + h)*128:...].
    wo_kb_tick = {}
    af = {i: 0 for i in range(8)}
    wl_count = {i: 0 for i in range(8)}
    af_loaded = set()
    wl_loaded = set()

    def emit_afbuf_load(half, kb, engine="sync"):
        if (half, kb) in af_loaded:
            return
        af_loaded.add((half, kb))
        h, c = kb // 8, kb % 8
        aslot = half * 4 + kb % 4
        war = wo_kb_tick.get((half, kb - 4))
        deps = [("cc", h + 1)]
        if war is not None:
            deps.append(("pe", war))
        af[aslot] += 16
        E(engine, dma(
            afbuf[aslot],
            cc_out[h][c * 128:(c + 1) * 128,
                      half * 1024:(half + 1) * 1024]),
          deps=deps, inc=f"af{aslot}", amt=16)

    def emit_woslab_load(half, kb, engine="sync"):
        if (half, kb) in wl_loaded:
            return
        wl_loaded.add((half, kb))
        h, c = kb // 8, kb % 8
        wslot = kb % 8
        sdeps = [("pe", wgrp[14])]   # wbuf[2] WAR (last qk reader)
        swar = wo_kb_tick.get((half, kb - 8))
        if swar is None and half == 1:
            swar = wo_kb_tick.get((0, kb + NKB - 8))
        if swar is not None:
            sdeps.append(("pe", swar))
        wl_count[wslot] += 16
        wrow = (c * NH + h) * 128
        E(engine, dma(
            woslab[:, wslot * 512:(wslot + 1) * 512],
            wo_ext[wrow:wrow + 128, :]),
          deps=sdeps, inc=f"wl{wslot}", amt=16)

    def emit_wo_loads(half, kb, engine="sync"):
        emit_afbuf_load(half, kb, engine)
        emit_woslab_load(half, kb, engine)

    exp_last = {}
    pv_read_end = {}
    state = {"stt": None, "recip": None, "bc": None, "bcast": None,
             "zmm": None, "adds": {}}
    stg_store = {}
    store_tick = {}
    ps_o_pair = {}
    ast = {0: 0, 1: 0, 2: 0, 3: 0}
    pending_zr = []
    pending_bc = []
    pending_adds = []
    adds_l1 = {}

    def finish_unit(d, bcast_tick):
        h, qp = d // 2, d % 2
        state["bc"] = E(
            "vector",
            lambda eng: eng.tensor_copy(bc_sb[:], pairs[3][:, 0:1024]),
            deps=[("pe", bcast_tick)], inc="dve")
        slot = d % 4
        sdeps = []
        if slot in stg_store:
            sdeps.append(stg_store[slot])

        def f_stt(eng, slot=slot, d=d):
            return eng.scalar_tensor_tensor(
                stg[slot][:], pairs[ps_o_pair[d]][:, 0:1024], 1.0, bc_sb[:],
                ALU.mult, ALU.mult)

        state["stt"] = E("vector", f_stt, deps=sdeps, inc="dve")

        sem = f"ast{slot}"
        ast[slot] += 16
        E("sync", dma(
            cc_in[h * 128:(h + 1) * 128, qp * 1024:(qp + 1) * 1024],
            stg[slot][:]),
            deps=[("dve", state["stt"])], inc=sem, amt=16)
        stg_store[slot] = (sem, ast[slot])
        store_tick[d] = (sem, ast[slot])

    def make_zr(d):
        def emit_zr():
            dps = [("dve", state["adds"][d]), ("dve", dve_ones)]
            if state["recip"] is not None:
                dps.append(("dve", state["recip"]))  # ps_z WAR
            if state["bc"] is not None:
                dps.append(("dve", state["bc"]))  # pairs[3] WAR vs bc copy
            for chf in range(2):
                bw = bank_war.get((3, chf))
                if bw is not None:
                    dps.append(("act", bw))  # pairs[3] WAR vs v(h1) evacs
            # Z = ones.T @ partials: contract the remaining 4 partial
            # slices on the PE (8 accumulating mms) instead of more DVE adds
            zmm = None
            for ch in range(4):
                for zc in range(2):
                    def fz(eng, ch=ch, zc=zc):
                        return eng.matmul(
                            pairs[3][0:1, zc * 512:(zc + 1) * 512], ones_col[:],
                            wbuf[0][:, ch * 1024 + zc * 512:
                                    ch * 1024 + (zc + 1) * 512],
                            start=(ch == 0), stop=(ch == 3))
                    last = ch == 3 and zc == 1
                    zmm = E("tensor", fz,
                            deps=dps if (ch == 0 and zc == 0) else (),
                            inc="pe" if last else None)
            state["zmm"] = zmm
            rdeps = [("pe", zmm)]
            if state["bcast"] is not None:
                rdeps.append(("pe", state["bcast"]))  # recip_bf WAR
            state["recip"] = E(
                "vector",
                lambda eng: eng.reciprocal(recip_bf[:], pairs[3][0:1, 0:1024]),
                deps=rdeps, inc="dve")
        return emit_zr

    def make_bcast(d):
        def emit_bcast():
            dps = [("dve", state["recip"])]
            if state["bc"] is not None:
                dps.append(("dve", state["bc"]))
            for chf in range(2):
                bw = bank_war.get((3, chf))
                if bw is not None:
                    dps.append(("act", bw))  # pairs[3] WAR vs v(h1) evacs
            bt = None
            for zc in range(2):
                def fb(eng, zc=zc):
                    return eng.matmul(
                        pairs[3][:, zc * 512:(zc + 1) * 512], ones_row[:],
                        recip_bf[:, zc * 512:(zc + 1) * 512],
                        start=True, stop=True)
                bt = E("tensor", fb, deps=dps if zc == 0 else (),
                       inc="pe" if zc == 1 else None)
            state["bcast"] = bt
            finish_unit(d, bt)
        return emit_bcast

    def emit_pass1(d, sks=None):
        h, qp = d // 2, d % 2
        base = (d % 2) * 16384
        if sks is None:
            sks = range(16)
        for sk in sks:
            pidx = sk % 2
            pair = pairs[pidx]
            deps = [("dve", rope_done[("q", h)]), ("dve", rope_done[("k", h)])]
            for chf in range(2):
                if bank_war.get((pidx, chf)) is not None:
                    deps.append(("act", bank_war[(pidx, chf)]))
            tick = None
            for qi in range(2):

                def mm1(eng, pair=pair, h=h, sk=sk, qp=qp, qi=qi):
                    return eng.matmul(
                        pair[:, qi * 512:(qi + 1) * 512],
                        kT_sb[:, h * S + sk * 128: h * S + sk * 128 + 128],
                        qT_sb[:, h * S + qp * 1024 + qi * 512:
                              h * S + qp * 1024 + (qi + 1) * 512],
                        start=True, stop=True)

                tick = E("tensor", mm1, deps=deps if qi == 0 else (),
                         inc="pe" if qi == 1 else None)

            if sk == 0 and pending_zr:
                pending_zr.pop(0)()
            if sk == 2 and pending_adds:
                pending_adds.pop(0)()
            if sk == 8 and pending_bc:
                pending_bc.pop(0)()

            edeps = [("pe", tick)]
            if d >= 2 and sk == 0:
                edeps.append(("pe", pv_read_end[d - 2]))
                edeps.append(("dve", adds_l1[d - 2]))
            if d < 2 and sk == 0:
                edeps.append(("pe", P_H0_END))  # arenaA WAR vs half0 x

            def f_exp(eng, pair=pair, base=base, sk=sk):
                return eng.activation(
                    arenaA[:, base + sk * 1024: base + (sk + 1) * 1024],
                    pair[:, 0:1024], AF.Exp, scale=SCALE)

            a = E("scalar", f_exp, deps=edeps, inc="act")
            exp_last[d] = a
            bank_war[(pidx, 0)] = a
            bank_war[(pidx, 1)] = a
        if 15 in sks:
            pending_adds.append(make_adds(d))

    def emit_pass2(d, opair=2):
        h, qp = d // 2, d % 2
        base = (d % 2) * 16384
        deps = [("act", exp_last[d])]
        if opair == 2 and state["stt"] is not None:
            deps.append(("dve", state["stt"]))
        # pair WAR + v_sb RAW vs the v(h1) evacs: units 0/1's exps precede
        # the v evacs on the act queue, so program order no longer covers
        # it. (3,1) is the last v evac; stale-but-harmless for later units.
        bw = bank_war.get((3, 1))
        if bw is not None:
            deps.append(("act", bw))
        for chf in range(2):
            bw = bank_war.get((opair, chf))
            if bw is not None:
                deps.append(("act", bw))
        tick = None
        for sk in range(16):
            for qi in range(2):

                def mpv(eng, sk=sk, h=h, base=base, qi=qi, opair=opair):
                    return eng.matmul(
                        pairs[opair][:, qi * 512:(qi + 1) * 512],
                        v_sb[:, sk * 512 + h * 128: sk * 512 + h * 128 + 128],
                        arenaA[:, base + sk * 1024 + qi * 512:
                               base + sk * 1024 + (qi + 1) * 512],
                        start=(sk == 0), stop=(sk == 15))

                tick = E("tensor", mpv,
                         deps=deps if (sk == 0 and qi == 0) else (),
                         inc="pe" if (sk == 15 and qi == 1) else None)
        pv_read_end[d] = tick
        ps_o_pair[d] = opair

        pending_zr.append(make_zr(d))
        pending_bc.append(make_bcast(d))

    def make_adds(d):
        # reduce 16 expT slices to 4 partials in wbuf[0] (dead after the
        # last qk group; the PE's zmm contracts the rest). Popped into the
        # NEXT pass1 window so the DVE queue never blocks the z-chain.
        base = (d % 2) * 16384

        def emit_adds():
            adeps = [("act", exp_last[d]), ("pe", wgrp[15])]
            if state["zmm"] is not None:
                adeps.append(("pe", state["zmm"]))  # scratch WAR vs zmm reads

            def fa(eng, q, acc, base=base):
                sl = arenaA[:, base + q * 4096:base + (q + 1) * 4096]
                if not acc:
                    return eng.tensor_add(
                        wbuf[0][:, 0:4096],
                        sl, arenaA[:, base + 4096 * (q + 1):
                                   base + 4096 * (q + 2)])
                return eng.tensor_add(wbuf[0][:, 0:4096],
                                      wbuf[0][:, 0:4096], sl)

            E("vector", lambda eng: fa(eng, 0, False), deps=adeps)
            E("vector", lambda eng: fa(eng, 2, True))
            tick = E("vector", lambda eng: fa(eng, 3, True), inc="dve")
            adds_l1[d] = state["adds"][d] = tick
        return emit_adds

    # PE order: qk(h0), v(h0), qk(h1)+rope+pass1(0,1), v(h1), attention
    for k in range(4):
        emit_slab(0, k)
    emit_qk_groups0()
    emit_v_groups(0)
    P_H0_END = v_end[0]
    for k in range(4):
        emit_slab(1, k)
    emit_qk_groups_h1()
    # ropes after all h1 groups: keeps the scalar queue's evac stream
    # unblocked (rope swap DMAs would HOL-block evacs mid-projection);
    # they drain during v(h1), well before pass1 needs them
    for hh in range(NH):
        emit_rope("q", qT_sb, hh)
        emit_rope("k", kT_sb, hh)
    emit_v_groups(1)

    emit_pass1(0)
    emit_pass1(1)
    for d in range(2, 8):
        emit_pass2(d - 2)
        emit_pass1(d)
        if d == 3:
            # woslab tiles for head 0 prefetch during early attention
            # (wbuf[2] WAR only — no collective dependency)
            for kb_pre in range(8):
                emit_woslab_load(0, kb_pre)
    # afbuf preloads sit on the sync queue BEFORE the last units' cc_in
    # stores; their cc(1) dep is satisfied mid-attention so they stream in
    # well before the wo matmuls need them
    for kb_pre in range(4):
        emit_afbuf_load(0, kb_pre)
    emit_pass2(6)             # PV(6); queues zr6/bc6
    pending_zr.pop(0)()       # zmm(6)+recip(6): reads ztree(6) before L1(7)
    pending_adds.pop(0)()     # tree(7), gated on zmm(6) via ztree WAR
    emit_pass2(7, opair=0)    # PV(7) -> pair0, overlaps unit 6's chain
    pending_bc.pop(0)()       # bcast(6)+stt(6)
    pending_zr.pop(0)()       # zmm(7)+recip(7)
    pending_bc.pop(0)()       # bcast(7)+stt(7)

    for h in range(NH):

        def f_ag(eng, h=h):
            return eng.collective_compute(
                "AllGather", ALU.bypass,
                replica_groups=[list(range(N_CORES))],
                ins=[cc_in[h * 128:(h + 1) * 128, :].opt()],
                outs=[cc_out[h][:].opt()])

        E("gpsimd", f_ag,
          deps=[store_tick[2 * h], store_tick[2 * h + 1]],
          inc="cc")

    # ---------------- wo projection ----------------
    y_evac_by_dout = {}
    for half in range(2):
        for kb in range(NKB):
            slot = kb % 4
            aslot = half * 4 + slot
            wslot = kb % 8
            emit_wo_loads(half, kb)

            mmdeps = [(f"af{aslot}", af[aslot]), (f"wl{wslot}", wl_count[wslot])]
            if kb == 0 and half == 0:
                mmdeps.append(("act", exp_last[7]))
                mmdeps.append(("dve", state["stt"]))
                mmdeps.append(("dve", state["recip"]))
            tick = None
            for dout in range(4):
                for c2 in range(2):
                    dd = mmdeps if (dout == 0 and c2 == 0) else []
                    if kb == 0 and half == 1 and c2 == 0:
                        # pairs[dout] WAR: only needs half-0's evacs of
                        # this dout, not the full evac+store tail
                        dd = list(dd) + [("act", y_evac_by_dout[dout])]

                    def mmo(eng, kb=kb, dout=dout, c2=c2,
                            aslot=aslot, wslot=wslot):
                        return eng.matmul(
                            pairs[dout][:, c2 * 512:(c2 + 1) * 512],
                            woslab[:, wslot * 512 + dout * 128:
                                   wslot * 512 + dout * 128 + 128],
                            afbuf[aslot][:, c2 * 512:(c2 + 1) * 512],
                            start=(kb == 0), stop=(kb == NKB - 1))

                    tick = E("tensor", mmo,
                             deps=dd,
                             inc="pe" if (dout == 3 and c2 == 1) else None)
            wo_kb_tick[(half, kb)] = tick

        wo_end = wo_kb_tick[(half, NKB - 1)]
        if half == 0:
            for kb_pre in range(4):
                emit_wo_loads(1, kb_pre, engine="scalar")
        for c2 in range(2):
            for dout in range(4):
                deps = [("pe", wo_end)]
                if half == 1:
                    deps.append(("yst", 16 * (c2 * 4 + dout + 1)))

                def evy(eng, dout=dout, c2=c2):
                    return eng.copy(
                        y_sb[:, (dout * 2 + c2) * 512:(dout * 2 + c2 + 1) * 512],
                        pairs[dout][:, c2 * 512:(c2 + 1) * 512])

                y_evac_last = E("scalar", evy, deps=deps, inc="act")
                if c2 == 1:
                    y_evac_by_dout[dout] = y_evac_last

                cbase = half * 1024 + c2 * 512
                E("sync", dma(
                    out_ext[dout * 128:(dout + 1) * 128, cbase:cbase + 512],
                    y_sb[:, (dout * 2 + c2) * 512:(dout * 2 + c2 + 1) * 512]),
                    deps=[("act", y_evac_last)], inc="yst", amt=16)

    E("sync", lambda eng: None, deps=[("yst", 256)])


# ======================= host side =======================

_NC_CACHE = None


def _get_nc():
    global _NC_CACHE
    if _NC_CACHE is None:
        _NC_CACHE = build()
    return _NC_CACHE


def _prep_inputs(x, freqs_cos, freqs_sin, wq, wk, wv, wo):
    bf = ml_dtypes.bfloat16
    x2 = np.asarray(x, np.float32).reshape(S, D)
    xT = np.ascontiguousarray(x2.T).astype(bf)
    perm = np.concatenate([np.arange(0, HD, 2), np.arange(1, HD, 2)])
    cos = np.asarray(freqs_cos, np.float32)
    sin = np.asarray(freqs_sin, np.float32)
    cosd = np.concatenate([cos.T, cos.T], axis=0).astype(bf)
    sins = np.concatenate([-sin.T, sin.T], axis=0).astype(bf)

    in_maps = []
    for c in range(N_CORES):
        cols_qk = np.concatenate([c * NL + h * HD + perm for h in range(NH)])
        cols_n = np.arange(c * NL, (c + 1) * NL)
        in_maps.append({
            "xT": xT,
            "wq": np.ascontiguousarray(np.asarray(wq, np.float32)[:, cols_qk]).astype(bf),
            "wk": np.ascontiguousarray(np.asarray(wk, np.float32)[:, cols_qk]).astype(bf),
            "wv": np.ascontiguousarray(np.asarray(wv, np.float32)[:, cols_n]).astype(bf),
            "wo": np.ascontiguousarray(np.asarray(wo, np.float32)[:, cols_n]).astype(bf),
            "cosd": cosd,
            "sins": sins,
        })
    return in_maps


def run(inputs, trace=False, **kw):
    nc = _get_nc()
    in_maps = _prep_inputs(**inputs)
    res = run_bass_kernel_spmd(nc, in_maps, core_ids=list(range(N_CORES)),
                               trace=trace, **kw)
    yT = np.concatenate([np.asarray(res.results[c]["out"], np.float32)
                         for c in range(N_CORES)], axis=0)
    out = np.ascontiguousarray(yT.T).reshape(1, S, D).astype(np.float32)
    return out, res


def kernel(x, freqs_cos, freqs_sin, wq, wk, wv, wo):
    out, _ = run(dict(x=x, freqs_cos=freqs_cos, freqs_sin=freqs_sin,
                      wq=wq, wk=wk, wv=wv, wo=wo))
    return out



# revision 84
# speedup vs baseline: 1.0692x; 1.0228x over previous
"""Distributed RoPE-attention kernel for 8 TRN2 NeuronCores (v3).

Sharding: tensor-parallel over heads (4 heads/core) for QKV+attention;
the attention output (bf16) is AllGather'd per local head (4 gathers,
fired as each head's two s-half units store, so heads 0-2 gather under
the remaining attention compute), then each core computes a 512-column
shard of the final wo projection, accumulating gathered head-blocks in
arrival order. Host concatenates the column shards — no all-reduce.

v4 over v2: per-head collectives (wo-tail stall ~100us -> ~0); bf16
broadcast matmuls and a bf16 1/Z (fp32 matmul is 4 cyc/row on the PE);
the softmax Z-path split DVE/PE (3 wide adds reduce 16 expT slices to
4 partials in wbuf0, the PE's ones-matmul contracts the rest); the
z-chain pops placed at sk0/2/8 of the pass1 windows so the in-order
DVE queue never blocks it; wo weight/activation tiles prefetched
during attention (woslab early with only a wbuf WAR, afbuf preloads
queued before the last cc_in stores); 4-slot wv slab ring; final store
split per (dout, s-chunk) with per-dout psum release for wo half 1.

Scheduling lessons baked in here: rope swap DMAs and exp slices must
NOT interleave with the projection evac stream on the scalar queue
(in-order HOL blocking stalls the PE's psum-WAR chain) — interleaving
the first attention units into the half-1 projections measured ~25us
SLOWER despite hiding two units of exp; attention matmuls run at the
mid p-state (~427ns/512col vs 256 sustained) because the exp lockstep
keeps resetting the PE's ramp.

Layouts are all "transposed" ([feature, seq]) so the PE never needs a
transpose: scoresT = kT.T @ qT, exp on ScalarE (PSUM->SBUF, bf16), PV
uses v as the stationary operand (outT = v.T @ expT), the softmax
denominator comes from a ones-column matmul, and the per-position 1/Z
broadcast is a K=1 outer-product matmul whose issue is deferred into
the next unit's matmul stream (keeps the reciprocal off the PE
critical path).

RoPE runs in an even/odd-permuted head basis (host permutes wq/wk
columns; q.k dot products are permutation invariant), which turns the
pair-swap into two 64-partition SBUF->SBUF DMAs plus three elementwise
DVE ops per head.

Raw bass (no Tile): this container's walrus rejects any instruction
with >1 attached sync-wait, so every dependency is an explicit
standalone wait_ge and semaphores are managed manually via cumulative
counters (one counter per buffer family / producer engine).
"""

import sys

sys.path.insert(0, "/opt/trn_rl_repo")

import numpy as np
import ml_dtypes

import concourse.bass as bass
import concourse.mybir as mybir
from concourse.bass_utils import run_bass_kernel_spmd
from concourse import bass_utils as _bu

_orig_run_command = _bu.run_command


def _patched_run_command(cmd, **kw):
    cmd = ["--enable-ldw-opt=true" if c == "--enable-ldw-opt=false" else c
           for c in cmd]
    return _orig_run_command(cmd, **kw)


_bu.run_command = _patched_run_command

N_CORES = 8
S = 2048
D = 4096
NH = 4            # local heads
HD = 128
NL = 512          # local feature columns
NKB = 32          # 128-row blocks over D
SCALE = 1.0 / float(np.sqrt(HD))

BF16 = mybir.dt.bfloat16
F32 = mybir.dt.float32
AF = mybir.ActivationFunctionType
ALU = mybir.AluOpType

ENGINES = ("sync", "tensor", "scalar", "vector", "gpsimd")
LAST_SCHED = None


class Sched:
    """Per-engine straight-line programs with cumulative-counter sems."""

    def __init__(self):
        self.prog = {e: [] for e in ENGINES}
        self.count = {}

    def emit(self, eng, fn, deps=(), inc=None, amt=1):
        dd = {}
        for sem, thr in deps:
            if thr is not None and thr > dd.get(sem, -1):
                dd[sem] = thr
        tick = None
        if inc is not None:
            tick = self.count.get(inc, 0) + amt
            self.count[inc] = tick
        self.prog[eng].append((fn, sorted(dd.items()), inc, amt))
        return tick

    def run(self, eng_name, eng, sems):
        observed = {}
        for fn, deps, inc, amt in self.prog[eng_name]:
            for sem, thr in deps:
                if observed.get(sem, 0) < thr:
                    eng.wait_ge(sems[sem], thr)
                    observed[sem] = thr
            inst = fn(eng)
            if inc is not None:
                inst.then_inc(sems[inc], amt)


def build():
    nc = bass.Bass(num_devices=N_CORES, dynamic_dma_scratch_size=64)

    xT_ext = nc.declare_dram_parameter("xT", [D, S], BF16, isOutput=False)
    wq_ext = nc.declare_dram_parameter("wq", [D, NL], BF16, isOutput=False)
    wk_ext = nc.declare_dram_parameter("wk", [D, NL], BF16, isOutput=False)
    wv_ext = nc.declare_dram_parameter("wv", [D, NL], BF16, isOutput=False)
    wo_ext = nc.declare_dram_parameter("wo", [D, NL], BF16, isOutput=False)
    cosd_ext = nc.declare_dram_parameter("cosd", [HD, S], BF16, isOutput=False)
    sins_ext = nc.declare_dram_parameter("sins", [HD, S], BF16, isOutput=False)
    out_ext = nc.declare_dram_parameter("out", [NL, S], F32, isOutput=True)

    cc_in = nc.dram_tensor("cc_in", [NL, S], BF16)
    cc_out = [
        nc.dram_tensor(f"cc_out{i}", [N_CORES * 128, S], BF16, addr_space="Shared")
        for i in range(NH)
    ]

    sem_names = [
        "xa", "xb", "wb0", "wb1", "wb2", "cs", "swp",
        "ast0", "ast1", "ast2", "ast3",
        "sl0", "sl1", "sl2", "sl3",
        "wl0", "wl1", "wl2", "wl3", "wl4", "wl5", "wl6", "wl7",
        "af0", "af1", "af2", "af3", "af4", "af5", "af6", "af7",
        "yst", "pe", "act", "dve", "cc",
    ]
    # (slab now has 4 slots; sl0..sl3 already declared)

    import contextlib

    with contextlib.ExitStack() as ctx:
        def sb(name, shape, dt):
            return ctx.enter_context(nc.sbuf_tensor(name, shape, dt))

        arenaA = sb("arenaA", [128, 32 * 1024], BF16)   # x half0 -> expT slabs
        arenaB = sb("arenaB", [128, 32 * 1024], BF16)   # x half1 -> afbuf/y_sb
        wbuf = [sb(f"wbuf{i}", [128, NKB * 128], BF16) for i in range(3)]
        slab = sb("slab", [128, 4 * 512], BF16)         # wv stream tiles
        qT_sb = sb("qT_sb", [128, NH * S], BF16)
        kT_sb = sb("kT_sb", [128, NH * S], BF16)
        v_sb = sb("v_sb", [128, 16 * 512], BF16)
        cosd_sb = sb("cosd_sb", [128, S], BF16)
        sins_sb = sb("sins_sb", [128, S], BF16)
        recip_sb = sb("recip", [1, 1024], F32)
        ones_col = sb("onesc", [128, 1], BF16)
        ones_row = sb("onesr", [1, 128], BF16)

        # aliases (temporal reuse, enforced by the schedule):
        qsw = wbuf[0][:, 0:2048]       # rope swap scratch (post q/k groups)
        t1 = wbuf[0][:, 2048:4096]
        t2 = wbuf[1][:, 0:2048]
        # attention-time scratch in arenaB's tail (x half1 dead by then)
        stg = [arenaB[:, 19456 + i * 1024: 19456 + (i + 1) * 1024]
               for i in range(4)]
        recip_bf = arenaB[0:1, 23552:24576]
        # attention normalize scratch aliases rope scratch (dead post-rope;
        # DVE program order separates the uses)
        bc_sb = arenaB[:, 17408:19456].bitcast(F32)  # [128, 1024] f32
        zacc = arenaB[:, 16384:17408]                # [128, 1024] bf16
        woslab = wbuf[2][:, 0:4096]    # 8 x [128,512] wo weight tiles
        afbuf = ([arenaB[:, i * 1024:(i + 1) * 1024] for i in range(4)] +
                 [arenaB[:, 12288 + i * 1024: 12288 + (i + 1) * 1024]
                  for i in range(4)])  # wo rhs, 4 slots per half
        y_sb = arenaB[:, 4096:12288].bitcast(F32)       # [128, 4096] f32

        pairs = [ctx.enter_context(nc.psum_tensor(f"pair{i}", [128, 1024], F32))
                 for i in range(4)]

        sems = {n: ctx.enter_context(nc.semaphore(n)) for n in sem_names}

        sch = Sched()
        global LAST_SCHED
        LAST_SCHED = sch
        _schedule(sch, locals())

        with nc.Block() as block:

            @block.sync
            def _(eng):
                sch.run("sync", eng, sems)

            @block.tensor
            def _(eng):
                sch.run("tensor", eng, sems)

            @block.scalar
            def _(eng):
                sch.run("scalar", eng, sems)

            @block.vector
            def _(eng):
                with nc.allow_low_precision(
                        reason="1/Z kept in bf16 for the broadcast matmul"):
                    sch.run("vector", eng, sems)

            @block.gpsimd
            def _(eng):
                sch.run("gpsimd", eng, sems)

    return nc


def _schedule(sch, env):
    def g(n):
        return env[n]

    xT_ext, wq_ext, wk_ext, wv_ext, wo_ext = (
        g("xT_ext"), g("wq_ext"), g("wk_ext"), g("wv_ext"), g("wo_ext"))
    cosd_ext, sins_ext, out_ext = g("cosd_ext"), g("sins_ext"), g("out_ext")
    cc_in, cc_out = g("cc_in"), g("cc_out")
    arenaA, arenaB, wbuf, slab = g("arenaA"), g("arenaB"), g("wbuf"), g("slab")
    qT_sb, kT_sb, v_sb = g("qT_sb"), g("kT_sb"), g("v_sb")
    cosd_sb, sins_sb = g("cosd_sb"), g("sins_sb")
    qsw, t1, t2 = g("qsw"), g("t1"), g("t2")
    stg, recip_sb, bc_sb = g("stg"), g("recip_sb"), g("bc_sb")
    recip_bf = g("recip_bf")
    zacc = g("zacc")
    woslab = g("woslab")
    ones_col, ones_row = g("ones_col"), g("ones_row")
    afbuf, y_sb = g("afbuf"), g("y_sb")
    pairs = g("pairs")
    arenas = [arenaA, arenaB]

    E = sch.emit

    def dma(out_ap, in_ap):
        return lambda eng: eng.dma_start(out=out_ap, in_=in_ap)

    # ---------------- SP: x loads (both halves up front) ----------------
    xsem = {}

    def emit_x_load(half, j):
        name = "xa" if half == 0 else "xb"
        xsem[half] = E("sync", dma(
            arenas[half][:, j * 8192:(j + 1) * 8192]
            .rearrange("p (kb s) -> p kb s", kb=8),
            xT_ext[j * 1024:(j + 1) * 1024, half * 1024:(half + 1) * 1024]
            .rearrange("(kb p) s -> p kb s", p=128)),
            inc=name, amt=16)

    # wb0 first so group 0 can start as soon as the first x quarter lands
    E("vector", lambda eng: eng.memset(ones_col[:], 1.0), inc="dve")
    dve_ones = E("vector", lambda eng: eng.memset(ones_row[:], 1.0), inc="dve")

    # ---------------- projections ----------------
    bank_war = {}      # (pair_idx, colhalf) -> act tick of last reader
    evac_tick = {}     # ("q"/"k", n, half) -> act tick
    wgrp = {}          # qk group idx -> pe tick of its last matmul
    vk_tick = {}       # (half, k) -> pe tick (for slab WAR)
    v_end = {}
    wb_tick = {}
    sl_count = {i: 0 for i in range(4)}

    GL1 = [0, 4, 1, 5, 2, 6, 3, 7]   # half-1 group order: q0 k0 q1 k1 ...

    def emit_qk_weight_dma(gg):
        half = gg // 8
        gl = GL1[gg - 8] if half == 1 else gg % 8
        t, n = ("q", gl) if gl < 4 else ("k", gl - 4)
        wext = wq_ext if t == "q" else wk_ext
        slot = gg % 3
        deps = []
        if wgrp.get(gg - 3) is not None:
            deps.append(("pe", wgrp[gg - 3]))
        wb_tick[gg] = E("sync", dma(
            wbuf[slot][:].rearrange("p (kb c) -> p kb c", kb=NKB),
            wext[:, n * 128:(n + 1) * 128].rearrange("(kb p) c -> p kb c", p=128)),
            deps=deps, inc=f"wb{slot}", amt=16)

    emit_x_load(0, 0)
    emit_qk_weight_dma(0)
    emit_qk_weight_dma(1)
    emit_x_load(0, 1)
    emit_x_load(0, 2)
    emit_x_load(0, 3)
    E("sync", dma(cosd_sb[:], cosd_ext[:]), inc="cs", amt=16)
    CS_ALL = E("sync", dma(sins_sb[:], sins_ext[:]), inc="cs", amt=16)
    preload_slabs = True

    def emit_one_qk_group(gg, half, gl, pidx):
        t, n = ("q", gl) if gl < 4 else ("k", gl - 4)
        slot = gg % 3
        pair = pairs[pidx]
        xname = "xa" if half == 0 else "xb"
        deps = [(f"wb{slot}", wb_tick[gg])]
        if gg >= 4:
            deps.append((xname, 64))
        else:
            deps.append((xname, 16))
        for chf in range(2):
            if bank_war.get((pidx, chf)) is not None:
                deps.append(("act", bank_war[(pidx, chf)]))
        tick = None
        for k in range(NKB):
            kdeps = ()
            if k == 0:
                kdeps = deps
            elif gg < 4 and k % 8 == 0:
                kdeps = [(xname, 16 * (k // 8 + 1))]
            for sc in range(2):
                last = (k == NKB - 1) and (sc == 1)

                def mm(eng, k=k, sc=sc, pair=pair, slot=slot, half=half):
                    return eng.matmul(
                        pair[:, sc * 512:(sc + 1) * 512],
                        wbuf[slot][:, k * 128:(k + 1) * 128],
                        arenas[half][:, k * 1024 + sc * 512:
                                     k * 1024 + (sc + 1) * 512],
                        start=(k == 0), stop=(k == NKB - 1))

                tick = E("tensor", mm,
                         deps=kdeps if sc == 0 else (),
                         inc="pe" if last else None)
        wgrp[gg] = tick
        dst = qT_sb if t == "q" else kT_sb

        def evac(eng, dst=dst, n=n, half=half, pair=pair):
            return eng.copy(
                dst[:, n * S + half * 1024: n * S + (half + 1) * 1024],
                pair[:, 0:1024])

        a = E("scalar", evac, deps=[("pe", tick)], inc="act")
        bank_war[(pidx, 0)] = a
        bank_war[(pidx, 1)] = a
        evac_tick[(t, n, half)] = a

    def emit_qk_groups0():
        for gl in range(8):
            emit_one_qk_group(gl, 0, gl, gl % 2)
            if gl + 2 <= 7:
                emit_qk_weight_dma(gl + 2)
            if gl in (1, 3, 5, 7):
                emit_x_load(1, (gl - 1) // 2)

    def emit_qk_groups_h1():
        # half-1 q/k groups in q0,k0,q1,k1,... order on psum pairs 2/3,
        # with per-head rope and the first two attention units' QK+exp
        # slices interleaved (their exps hide under the projection PE work)
        for p in range(8):
            gg = 8 + p
            emit_one_qk_group(gg, 1, GL1[p], 2 + p % 2)
            if gg + 3 <= 15:
                emit_qk_weight_dma(gg + 3)
            pass

    def emit_slab(half, k):
        slot = k % 4
        war = vk_tick.get((half, k - 4))
        if war is None and half == 1:
            war = vk_tick.get((0, k + NKB - 4))
        deps = [("pe", war)] if war is not None else []
        sl_count[slot] += 16
        E("sync", dma(
            slab[:, slot * 512:(slot + 1) * 512],
            wv_ext[k * 128:(k + 1) * 128, :]),
          deps=deps, inc=f"sl{slot}", amt=16)

    def emit_v_groups(half):
        tick = None
        for k in range(NKB):
            slot = k % 4
            deps = [(f"sl{slot}", 16 * (half * 8 + k // 4 + 1))]
            if k == 0:
                for pidx in range(4):
                    for chf in range(2):
                        if bank_war.get((pidx, chf)) is not None:
                            deps.append(("act", bank_war[(pidx, chf)]))
            for st in range(8):
                last = st == 7

                def mmv(eng, k=k, st=st, half=half, slot=slot):
                    return eng.matmul(
                        pairs[st // 2][:, (st % 2) * 512:(st % 2 + 1) * 512],
                        arenas[half][:, k * 1024 + st * 128:
                                     k * 1024 + st * 128 + 128],
                        slab[:, slot * 512:(slot + 1) * 512],
                        start=(k == 0), stop=(k == NKB - 1))

                tick = E("tensor", mmv, deps=deps if st == 0 else (),
                         inc="pe" if last else None)
            vk_tick[(half, k)] = tick
            if k + 4 < NKB:
                emit_slab(half, k + 4)
            if half == 0 and k < 2:
                emit_qk_weight_dma(8 + k)
            if half == 0 and k == 2:
                emit_qk_weight_dma(10)
        v_end[half] = tick
        for st in range(8):
            stg_idx = half * 8 + st

            def evacv(eng, stg_idx=stg_idx, st=st):
                return eng.copy(
                    v_sb[:, stg_idx * 512:(stg_idx + 1) * 512],
                    pairs[st // 2][:, (st % 2) * 512:(st % 2 + 1) * 512])

            evdeps = [("pe", v_end[half])]
            if half == 1:
                evdeps.append(("dve", rope_last))  # v_sb rope-scratch WAR
            a = E("scalar", evacv, deps=evdeps, inc="act")
            bank_war[(st // 2, st % 2)] = a

    # ---------------- RoPE (in-place, v_sb tail scratch) ----------------
    swp = 0
    prev_sw = None
    rope_last = None
    rope_done = {}
    rp_t1 = v_sb[:, 4096:6144]   # v(h1) region: free until v(h1) evacs

    def emit_rope(t, tsb, h):
        nonlocal swp, prev_sw, rope_last
        c0 = h * S
        d0 = [("act", evac_tick[(t, h, 0)]), ("act", evac_tick[(t, h, 1)])]
        dsw = d0 + ([("dve", prev_sw)] if prev_sw is not None else [])
        swp = E("scalar", dma(v_sb[0:64, 6144:8192],
                              tsb[64:128, c0:c0 + S]),
                deps=dsw, inc="swp", amt=16)
        swp = E("scalar", dma(v_sb[64:128, 6144:8192],
                              tsb[0:64, c0:c0 + S]), inc="swp", amt=16)

        def f_t1(eng, tsb=tsb, c0=c0):
            return eng.tensor_mul(rp_t1, tsb[:, c0:c0 + S], cosd_sb[:])

        E("vector", f_t1, deps=d0 + [("cs", CS_ALL)], inc="dve")

        def f_t2(eng, tsb=tsb, c0=c0):
            return eng.tensor_mul(tsb[:, c0:c0 + S], v_sb[:, 6144:8192],
                                  sins_sb[:])

        prev_sw = E("vector", f_t2, deps=[("swp", swp)], inc="dve")

        def f_add(eng, tsb=tsb, c0=c0):
            return eng.tensor_add(tsb[:, c0:c0 + S], tsb[:, c0:c0 + S],
                                  rp_t1)

        rope_last = rope_done[(t, h)] = E("vector", f_add, inc="dve")

    # ------------- attention: 8 half-units (head-major) -------------
    # dunit d = h*2 + qp covers head h, s-half qp (two sq quarters).
    # expT slab (d%2) = arenaA[:, (d%2)*16384 : +16384] as [16 sk][1024].
    # AllGather is per local head (4 gathers): gather(h) fires as soon as
    # units 2h, 2h+1 have stored, so gathers h0-h2 hide under attention.
    # wo kb enumerates (head, core): kb = h*8 + c -> gathered rows
    # cc_out[h][c*128:...], weight rows wo_ext[(c*4 + h)*128:...].
    wo_kb_tick = {}
    af = {i: 0 for i in range(8)}
    wl_count = {i: 0 for i in range(8)}
    af_loaded = set()
    wl_loaded = set()

    def emit_afbuf_load(half, kb, engine="sync"):
        if (half, kb) in af_loaded:
            return
        af_loaded.add((half, kb))
        h, c = kb // 8, kb % 8
        aslot = half * 4 + kb % 4
        war = wo_kb_tick.get((half, kb - 4))
        deps = [("cc", h + 1)]
        if war is not None:
            deps.append(("pe", war))
        af[aslot] += 16
        E(engine, dma(
            afbuf[aslot],
            cc_out[h][c * 128:(c + 1) * 128,
                      half * 1024:(half + 1) * 1024]),
          deps=deps, inc=f"af{aslot}", amt=16)

    def emit_woslab_load(half, kb, engine="sync"):
        if (half, kb) in wl_loaded:
            return
        wl_loaded.add((half, kb))
        h, c = kb // 8, kb % 8
        wslot = kb % 8
        sdeps = [("pe", wgrp[14])]   # wbuf[2] WAR (last qk reader)
        swar = wo_kb_tick.get((half, kb - 8))
        if swar is None and half == 1:
            swar = wo_kb_tick.get((0, kb + NKB - 8))
        if swar is not None:
            sdeps.append(("pe", swar))
        wl_count[wslot] += 16
        wrow = (c * NH + h) * 128
        E(engine, dma(
            woslab[:, wslot * 512:(wslot + 1) * 512],
            wo_ext[wrow:wrow + 128, :]),
          deps=sdeps, inc=f"wl{wslot}", amt=16)

    def emit_wo_loads(half, kb, engine="sync"):
        emit_afbuf_load(half, kb, engine)
        emit_woslab_load(half, kb, engine)

    exp_last = {}
    pv_read_end = {}
    state = {"stt": None, "recip": None, "bc": None, "bcast": None,
             "zmm": None, "adds": {}}
    stg_store = {}
    store_tick = {}
    ps_o_pair = {}
    ast = {0: 0, 1: 0, 2: 0, 3: 0}
    pending_zr = []
    pending_bc = []
    pending_adds = []
    adds_l1 = {}

    def finish_unit(d, bcast_tick):
        h, qp = d // 2, d % 2
        state["bc"] = E(
            "vector",
            lambda eng: eng.tensor_copy(bc_sb[:], pairs[3][:, 0:1024]),
            deps=[("pe", bcast_tick)], inc="dve")
        slot = d % 4
        sdeps = []
        if slot in stg_store:
            sdeps.append(stg_store[slot])

        def f_stt(eng, slot=slot, d=d):
            return eng.scalar_tensor_tensor(
                stg[slot][:], pairs[ps_o_pair[d]][:, 0:1024], 1.0, bc_sb[:],
                ALU.mult, ALU.mult)

        state["stt"] = E("vector", f_stt, deps=sdeps, inc="dve")

        sem = f"ast{slot}"
        ast[slot] += 16
        E("sync", dma(
            cc_in[h * 128:(h + 1) * 128, qp * 1024:(qp + 1) * 1024],
            stg[slot][:]),
            deps=[("dve", state["stt"])], inc=sem, amt=16)
        stg_store[slot] = (sem, ast[slot])
        store_tick[d] = (sem, ast[slot])

    def make_zr(d):
        def emit_zr():
            dps = [("dve", state["adds"][d]), ("dve", dve_ones)]
            if state["recip"] is not None:
                dps.append(("dve", state["recip"]))  # ps_z WAR
            if state["bc"] is not None:
                dps.append(("dve", state["bc"]))  # pairs[3] WAR vs bc copy
            for chf in range(2):
                bw = bank_war.get((3, chf))
                if bw is not None:
                    dps.append(("act", bw))  # pairs[3] WAR vs v(h1) evacs
            # Z = ones.T @ partials: contract the remaining 4 partial
            # slices on the PE (8 accumulating mms) instead of more DVE adds
            zmm = None
            for ch in range(4):
                for zc in range(2):
                    def fz(eng, ch=ch, zc=zc):
                        return eng.matmul(
                            pairs[3][0:1, zc * 512:(zc + 1) * 512], ones_col[:],
                            wbuf[0][:, ch * 1024 + zc * 512:
                                    ch * 1024 + (zc + 1) * 512],
                            start=(ch == 0), stop=(ch == 3))
                    last = ch == 3 and zc == 1
                    zmm = E("tensor", fz,
                            deps=dps if (ch == 0 and zc == 0) else (),
                            inc="pe" if last else None)
            state["zmm"] = zmm
            rdeps = [("pe", zmm)]
            if state["bcast"] is not None:
                rdeps.append(("pe", state["bcast"]))  # recip_bf WAR
            state["recip"] = E(
                "vector",
                lambda eng: eng.reciprocal(recip_bf[:], pairs[3][0:1, 0:1024]),
                deps=rdeps, inc="dve")
        return emit_zr

    def make_bcast(d):
        def emit_bcast():
            dps = [("dve", state["recip"])]
            if state["bc"] is not None:
                dps.append(("dve", state["bc"]))
            for chf in range(2):
                bw = bank_war.get((3, chf))
                if bw is not None:
                    dps.append(("act", bw))  # pairs[3] WAR vs v(h1) evacs
            bt = None
            for zc in range(2):
                def fb(eng, zc=zc):
                    return eng.matmul(
                        pairs[3][:, zc * 512:(zc + 1) * 512], ones_row[:],
                        recip_bf[:, zc * 512:(zc + 1) * 512],
                        start=True, stop=True)
                bt = E("tensor", fb, deps=dps if zc == 0 else (),
                       inc="pe" if zc == 1 else None)
            state["bcast"] = bt
            finish_unit(d, bt)
        return emit_bcast

    def emit_pass1(d, sks=None, filler=None):
        h, qp = d // 2, d % 2
        base = (d % 2) * 16384
        fill_ticks = {}
        if sks is None:
            sks = range(16)
        for sk in sks:
            pidx = sk % 2
            pair = pairs[pidx]
            deps = [("dve", rope_done[("q", h)]), ("dve", rope_done[("k", h)])]
            for chf in range(2):
                if bank_war.get((pidx, chf)) is not None:
                    deps.append(("act", bank_war[(pidx, chf)]))
            tick = None
            for qi in range(2):

                def mm1(eng, pair=pair, h=h, sk=sk, qp=qp, qi=qi):
                    return eng.matmul(
                        pair[:, qi * 512:(qi + 1) * 512],
                        kT_sb[:, h * S + sk * 128: h * S + sk * 128 + 128],
                        qT_sb[:, h * S + qp * 1024 + qi * 512:
                              h * S + qp * 1024 + (qi + 1) * 512],
                        start=True, stop=True)

                tick = E("tensor", mm1, deps=deps if qi == 0 else (),
                         inc="pe" if qi == 1 else None)

            if sk == 0 and pending_zr:
                pending_zr.pop(0)()
            if sk == 2 and pending_adds:
                pending_adds.pop(0)()
            if sk == 14 and pending_bc:
                # sk14 (not 8): the interleaved PV(d-2) finishes at sk11, and
                # stt(d-2) must read its psum only after that
                pending_bc.pop(0)()

            # interleave PV(d-2) slices between QK pairs: they fill the
            # exp-lockstep gaps on the PE. Emitted BEFORE this sk's exp so
            # the exp's per-slice slab WAR can reference the PV tick.
            if filler is not None:
                if sk < 11:
                    fill_ticks[sk] = filler(sk)
                elif sk == 11:
                    for s2 in range(11, 16):
                        fill_ticks[s2] = filler(s2)

            edeps = [("pe", tick)]
            if d >= 2 and sk == 0:
                edeps.append(("dve", adds_l1[d - 2]))
            if filler is not None:
                # exp(d, sk) overwrites the slab slice PV(d-2, sk) reads
                edeps.append(("pe", fill_ticks[sk]))
            if d < 2 and sk == 0:
                edeps.append(("pe", P_H0_END))  # arenaA WAR vs half0 x

            def f_exp(eng, pair=pair, base=base, sk=sk):
                return eng.activation(
                    arenaA[:, base + sk * 1024: base + (sk + 1) * 1024],
                    pair[:, 0:1024], AF.Exp, scale=SCALE)

            a = E("scalar", f_exp, deps=edeps, inc="act")
            exp_last[d] = a
            bank_war[(pidx, 0)] = a
            bank_war[(pidx, 1)] = a
        if 15 in sks:
            pending_adds.append(make_adds(d))

    def begin_pv(d, opair=2):
        # slice-granular PV emitter so PV(d) can interleave into the NEXT
        # unit's QK stream, filling the exp-lockstep gaps with real work
        h = d // 2
        base = (d % 2) * 16384
        deps = [("act", exp_last[d])]
        if opair == 2 and state["stt"] is not None:
            deps.append(("dve", state["stt"]))
        # pair WAR + v_sb RAW vs the v(h1) evacs: units 0/1's exps precede
        # the v evacs on the act queue, so program order no longer covers
        # it. (3,1) is the last v evac; stale-but-harmless for later units.
        bw = bank_war.get((3, 1))
        if bw is not None:
            deps.append(("act", bw))
        for chf in range(2):
            bw = bank_war.get((opair, chf))
            if bw is not None:
                deps.append(("act", bw))
        ps_o_pair[d] = opair
        pending_zr.append(make_zr(d))
        pending_bc.append(make_bcast(d))

        def emit_pv_slice(sk):
            tick = None
            for qi in range(2):

                def mpv(eng, sk=sk, h=h, base=base, qi=qi, opair=opair):
                    return eng.matmul(
                        pairs[opair][:, qi * 512:(qi + 1) * 512],
                        v_sb[:, sk * 512 + h * 128: sk * 512 + h * 128 + 128],
                        arenaA[:, base + sk * 1024 + qi * 512:
                               base + sk * 1024 + (qi + 1) * 512],
                        start=(sk == 0), stop=(sk == 15))

                tick = E("tensor", mpv,
                         deps=deps if (sk == 0 and qi == 0) else (),
                         inc="pe" if qi == 1 else None)
            if sk == 15:
                pv_read_end[d] = tick
            return tick
        return emit_pv_slice

    def emit_pass2(d, opair=2):
        pv = begin_pv(d, opair)
        for sk in range(16):
            pv(sk)

    def make_adds(d):
        # reduce 16 expT slices to 4 partials in wbuf[0] (dead after the
        # last qk group; the PE's zmm contracts the rest). Popped into the
        # NEXT pass1 window so the DVE queue never blocks the z-chain.
        base = (d % 2) * 16384

        def emit_adds():
            adeps = [("act", exp_last[d]), ("pe", wgrp[15])]
            if state["zmm"] is not None:
                adeps.append(("pe", state["zmm"]))  # scratch WAR vs zmm reads

            def fa(eng, q, acc, base=base):
                sl = arenaA[:, base + q * 4096:base + (q + 1) * 4096]
                if not acc:
                    return eng.tensor_add(
                        wbuf[0][:, 0:4096],
                        sl, arenaA[:, base + 4096 * (q + 1):
                                   base + 4096 * (q + 2)])
                return eng.tensor_add(wbuf[0][:, 0:4096],
                                      wbuf[0][:, 0:4096], sl)

            E("vector", lambda eng: fa(eng, 0, False), deps=adeps)
            E("vector", lambda eng: fa(eng, 2, True))
            tick = E("vector", lambda eng: fa(eng, 3, True), inc="dve")
            adds_l1[d] = state["adds"][d] = tick
        return emit_adds

    # PE order: qk(h0), v(h0), qk(h1)+rope+pass1(0,1), v(h1), attention
    for k in range(4):
        emit_slab(0, k)
    emit_qk_groups0()
    emit_v_groups(0)
    P_H0_END = v_end[0]
    for k in range(4):
        emit_slab(1, k)
    emit_qk_groups_h1()
    # ropes after all h1 groups: keeps the scalar queue's evac stream
    # unblocked (rope swap DMAs would HOL-block evacs mid-projection);
    # they drain during v(h1), well before pass1 needs them
    for hh in range(NH):
        emit_rope("q", qT_sb, hh)
        emit_rope("k", kT_sb, hh)
    emit_v_groups(1)

    emit_pass1(0)
    emit_pass1(1)
    for d in range(2, 8):
        emit_pass1(d, filler=begin_pv(d - 2))
        if d == 3:
            # woslab tiles for head 0 prefetch during early attention
            # (wbuf[2] WAR only — no collective dependency)
            for kb_pre in range(8):
                emit_woslab_load(0, kb_pre)
    # afbuf preloads sit on the sync queue BEFORE the last units' cc_in
    # stores; their cc(1) dep is satisfied mid-attention so they stream in
    # well before the wo matmuls need them
    for kb_pre in range(4):
        emit_afbuf_load(0, kb_pre)
    emit_pass2(6)             # PV(6); queues zr6/bc6
    pending_zr.pop(0)()       # zmm(6)+recip(6): reads ztree(6) before L1(7)
    pending_adds.pop(0)()     # tree(7), gated on zmm(6) via ztree WAR
    emit_pass2(7, opair=0)    # PV(7) -> pair0, overlaps unit 6's chain
    pending_bc.pop(0)()       # bcast(6)+stt(6)
    pending_zr.pop(0)()       # zmm(7)+recip(7)
    pending_bc.pop(0)()       # bcast(7)+stt(7)

    for h in range(NH):

        def f_ag(eng, h=h):
            return eng.collective_compute(
                "AllGather", ALU.bypass,
                replica_groups=[list(range(N_CORES))],
                ins=[cc_in[h * 128:(h + 1) * 128, :].opt()],
                outs=[cc_out[h][:].opt()])

        E("gpsimd", f_ag,
          deps=[store_tick[2 * h], store_tick[2 * h + 1]],
          inc="cc")

    # ---------------- wo projection ----------------
    y_evac_by_dout = {}
    for half in range(2):
        for kb in range(NKB):
            slot = kb % 4
            aslot = half * 4 + slot
            wslot = kb % 8
            emit_wo_loads(half, kb)

            mmdeps = [(f"af{aslot}", af[aslot]), (f"wl{wslot}", wl_count[wslot])]
            if kb == 0 and half == 0:
                mmdeps.append(("act", exp_last[7]))
                mmdeps.append(("dve", state["stt"]))
                mmdeps.append(("dve", state["recip"]))
            tick = None
            for dout in range(4):
                for c2 in range(2):
                    dd = mmdeps if (dout == 0 and c2 == 0) else []
                    if kb == 0 and half == 1 and c2 == 0:
                        # pairs[dout] WAR: only needs half-0's evacs of
                        # this dout, not the full evac+store tail
                        dd = list(dd) + [("act", y_evac_by_dout[dout])]

                    def mmo(eng, kb=kb, dout=dout, c2=c2,
                            aslot=aslot, wslot=wslot):
                        return eng.matmul(
                            pairs[dout][:, c2 * 512:(c2 + 1) * 512],
                            woslab[:, wslot * 512 + dout * 128:
                                   wslot * 512 + dout * 128 + 128],
                            afbuf[aslot][:, c2 * 512:(c2 + 1) * 512],
                            start=(kb == 0), stop=(kb == NKB - 1))

                    tick = E("tensor", mmo,
                             deps=dd,
                             inc="pe" if (dout == 3 and c2 == 1) else None)
            wo_kb_tick[(half, kb)] = tick

        wo_end = wo_kb_tick[(half, NKB - 1)]
        if half == 0:
            for kb_pre in range(4):
                emit_wo_loads(1, kb_pre, engine="scalar")
        for c2 in range(2):
            for dout in range(4):
                deps = [("pe", wo_end)]
                if half == 1:
                    deps.append(("yst", 16 * (c2 * 4 + dout + 1)))

                def evy(eng, dout=dout, c2=c2):
                    return eng.copy(
                        y_sb[:, (dout * 2 + c2) * 512:(dout * 2 + c2 + 1) * 512],
                        pairs[dout][:, c2 * 512:(c2 + 1) * 512])

                y_evac_last = E("scalar", evy, deps=deps, inc="act")
                if c2 == 1:
                    y_evac_by_dout[dout] = y_evac_last

                cbase = half * 1024 + c2 * 512
                E("sync", dma(
                    out_ext[dout * 128:(dout + 1) * 128, cbase:cbase + 512],
                    y_sb[:, (dout * 2 + c2) * 512:(dout * 2 + c2 + 1) * 512]),
                    deps=[("act", y_evac_last)], inc="yst", amt=16)

    E("sync", lambda eng: None, deps=[("yst", 256)])


# ======================= host side =======================

_NC_CACHE = None


def _get_nc():
    global _NC_CACHE
    if _NC_CACHE is None:
        _NC_CACHE = build()
    return _NC_CACHE


def _prep_inputs(x, freqs_cos, freqs_sin, wq, wk, wv, wo):
    bf = ml_dtypes.bfloat16
    x2 = np.asarray(x, np.float32).reshape(S, D)
    xT = np.ascontiguousarray(x2.T).astype(bf)
    perm = np.concatenate([np.arange(0, HD, 2), np.arange(1, HD, 2)])
    cos = np.asarray(freqs_cos, np.float32)
    sin = np.asarray(freqs_sin, np.float32)
    cosd = np.concatenate([cos.T, cos.T], axis=0).astype(bf)
    sins = np.concatenate([-sin.T, sin.T], axis=0).astype(bf)

    in_maps = []
    for c in range(N_CORES):
        cols_qk = np.concatenate([c * NL + h * HD + perm for h in range(NH)])
        cols_n = np.arange(c * NL, (c + 1) * NL)
        in_maps.append({
            "xT": xT,
            "wq": np.ascontiguousarray(np.asarray(wq, np.float32)[:, cols_qk]).astype(bf),
            "wk": np.ascontiguousarray(np.asarray(wk, np.float32)[:, cols_qk]).astype(bf),
            "wv": np.ascontiguousarray(np.asarray(wv, np.float32)[:, cols_n]).astype(bf),
            "wo": np.ascontiguousarray(np.asarray(wo, np.float32)[:, cols_n]).astype(bf),
            "cosd": cosd,
            "sins": sins,
        })
    return in_maps


def run(inputs, trace=False, **kw):
    nc = _get_nc()
    in_maps = _prep_inputs(**inputs)
    res = run_bass_kernel_spmd(nc, in_maps, core_ids=list(range(N_CORES)),
                               trace=trace, **kw)
    yT = np.concatenate([np.asarray(res.results[c]["out"], np.float32)
                         for c in range(N_CORES)], axis=0)
    out = np.ascontiguousarray(yT.T).reshape(1, S, D).astype(np.float32)
    return out, res


def kernel(x, freqs_cos, freqs_sin, wq, wk, wv, wo):
    out, _ = run(dict(x=x, freqs_cos=freqs_cos, freqs_sin=freqs_sin,
                      wq=wq, wk=wk, wv=wv, wo=wo))
    return out



# revision 88
# speedup vs baseline: 1.0705x; 1.0012x over previous
"""Distributed RoPE-attention kernel for 8 TRN2 NeuronCores (v3).

Sharding: tensor-parallel over heads (4 heads/core) for QKV+attention;
the attention output (bf16) is AllGather'd per local head (4 gathers,
fired as each head's two s-half units store, so heads 0-2 gather under
the remaining attention compute), then each core computes a 512-column
shard of the final wo projection, accumulating gathered head-blocks in
arrival order. Host concatenates the column shards — no all-reduce.

v4 over v2: per-head collectives (wo-tail stall ~100us -> ~0); bf16
broadcast matmuls and a bf16 1/Z (fp32 matmul is 4 cyc/row on the PE);
the softmax Z-path split DVE/PE (3 wide adds reduce 16 expT slices to
4 partials in wbuf0, the PE's ones-matmul contracts the rest); the
z-chain pops placed at sk0/2/8 of the pass1 windows so the in-order
DVE queue never blocks it; wo weight/activation tiles prefetched
during attention (woslab early with only a wbuf WAR, afbuf preloads
queued before the last cc_in stores); 4-slot wv slab ring; final store
split per (dout, s-chunk) with per-dout psum release for wo half 1;
PV(d-2) slices interleaved 1:1 into pass1(d)'s QK stream (dense tail
at sk11 so stt(d-2) can pop at sk14), with a per-slice exp->PV slab
WAR — the PV work fills the exp-lockstep gaps the PE otherwise idles
in (~19us).

Scheduling lessons baked in here: rope swap DMAs and exp slices must
NOT interleave with the projection evac stream on the scalar queue
(in-order HOL blocking stalls the PE's psum-WAR chain) — interleaving
the first attention units into the half-1 projections measured ~25us
SLOWER despite hiding two units of exp; attention matmuls run at the
mid p-state (~427ns/512col vs 256 sustained) because the exp lockstep
keeps resetting the PE's ramp.

Layouts are all "transposed" ([feature, seq]) so the PE never needs a
transpose: scoresT = kT.T @ qT, exp on ScalarE (PSUM->SBUF, bf16), PV
uses v as the stationary operand (outT = v.T @ expT), the softmax
denominator comes from a ones-column matmul, and the per-position 1/Z
broadcast is a K=1 outer-product matmul whose issue is deferred into
the next unit's matmul stream (keeps the reciprocal off the PE
critical path).

RoPE runs in an even/odd-permuted head basis (host permutes wq/wk
columns; q.k dot products are permutation invariant), which turns the
pair-swap into two 64-partition SBUF->SBUF DMAs plus three elementwise
DVE ops per head.

Raw bass (no Tile): this container's walrus rejects any instruction
with >1 attached sync-wait, so every dependency is an explicit
standalone wait_ge and semaphores are managed manually via cumulative
counters (one counter per buffer family / producer engine).
"""

import sys

sys.path.insert(0, "/opt/trn_rl_repo")

import numpy as np
import ml_dtypes

import concourse.bass as bass
import concourse.mybir as mybir
from concourse.bass_utils import run_bass_kernel_spmd
from concourse import bass_utils as _bu

_orig_run_command = _bu.run_command


def _patched_run_command(cmd, **kw):
    cmd = ["--enable-ldw-opt=true" if c == "--enable-ldw-opt=false" else c
           for c in cmd]
    return _orig_run_command(cmd, **kw)


_bu.run_command = _patched_run_command

N_CORES = 8
S = 2048
D = 4096
NH = 4            # local heads
HD = 128
NL = 512          # local feature columns
NKB = 32          # 128-row blocks over D
SCALE = 1.0 / float(np.sqrt(HD))

BF16 = mybir.dt.bfloat16
F32 = mybir.dt.float32
AF = mybir.ActivationFunctionType
ALU = mybir.AluOpType

ENGINES = ("sync", "tensor", "scalar", "vector", "gpsimd")
LAST_SCHED = None


class Sched:
    """Per-engine straight-line programs with cumulative-counter sems."""

    def __init__(self):
        self.prog = {e: [] for e in ENGINES}
        self.count = {}

    def emit(self, eng, fn, deps=(), inc=None, amt=1):
        dd = {}
        for sem, thr in deps:
            if thr is not None and thr > dd.get(sem, -1):
                dd[sem] = thr
        tick = None
        if inc is not None:
            tick = self.count.get(inc, 0) + amt
            self.count[inc] = tick
        self.prog[eng].append((fn, sorted(dd.items()), inc, amt))
        return tick

    def run(self, eng_name, eng, sems):
        observed = {}
        for fn, deps, inc, amt in self.prog[eng_name]:
            for sem, thr in deps:
                if observed.get(sem, 0) < thr:
                    eng.wait_ge(sems[sem], thr)
                    observed[sem] = thr
            inst = fn(eng)
            if inc is not None:
                inst.then_inc(sems[inc], amt)


def build():
    nc = bass.Bass(num_devices=N_CORES, dynamic_dma_scratch_size=64)

    xT_ext = nc.declare_dram_parameter("xT", [D, S], BF16, isOutput=False)
    wq_ext = nc.declare_dram_parameter("wq", [D, NL], BF16, isOutput=False)
    wk_ext = nc.declare_dram_parameter("wk", [D, NL], BF16, isOutput=False)
    wv_ext = nc.declare_dram_parameter("wv", [D, NL], BF16, isOutput=False)
    wo_ext = nc.declare_dram_parameter("wo", [D, NL], BF16, isOutput=False)
    cosd_ext = nc.declare_dram_parameter("cosd", [HD, S], BF16, isOutput=False)
    sins_ext = nc.declare_dram_parameter("sins", [HD, S], BF16, isOutput=False)
    out_ext = nc.declare_dram_parameter("out", [NL, S], F32, isOutput=True)

    cc_in = nc.dram_tensor("cc_in", [NL, S], BF16)
    cc_out = [
        nc.dram_tensor(f"cc_out{i}", [N_CORES * 128, S], BF16, addr_space="Shared")
        for i in range(NH)
    ]

    sem_names = [
        "xa", "xb", "wb0", "wb1", "wb2", "cs", "swp",
        "ast0", "ast1", "ast2", "ast3",
        "sl0", "sl1", "sl2", "sl3",
        "wl0", "wl1", "wl2", "wl3", "wl4", "wl5", "wl6", "wl7",
        "af0", "af1", "af2", "af3", "af4", "af5", "af6", "af7",
        "yst", "pe", "act", "dve", "cc",
    ]
    # (slab now has 4 slots; sl0..sl3 already declared)

    import contextlib

    with contextlib.ExitStack() as ctx:
        def sb(name, shape, dt):
            return ctx.enter_context(nc.sbuf_tensor(name, shape, dt))

        arenaA = sb("arenaA", [128, 32 * 1024], BF16)   # x half0 -> expT slabs
        arenaB = sb("arenaB", [128, 32 * 1024], BF16)   # x half1 -> afbuf/y_sb
        wbuf = [sb(f"wbuf{i}", [128, NKB * 128], BF16) for i in range(3)]
        slab = sb("slab", [128, 4 * 512], BF16)         # wv stream tiles
        qT_sb = sb("qT_sb", [128, NH * S], BF16)
        kT_sb = sb("kT_sb", [128, NH * S], BF16)
        v_sb = sb("v_sb", [128, 16 * 512], BF16)
        cosd_sb = sb("cosd_sb", [128, S], BF16)
        sins_sb = sb("sins_sb", [128, S], BF16)
        recip_sb = sb("recip", [1, 1024], F32)
        ones_col = sb("onesc", [128, 1], BF16)
        ones_row = sb("onesr", [1, 128], BF16)

        # aliases (temporal reuse, enforced by the schedule):
        qsw = wbuf[0][:, 0:2048]       # rope swap scratch (post q/k groups)
        t1 = wbuf[0][:, 2048:4096]
        t2 = wbuf[1][:, 0:2048]
        # attention-time scratch in arenaB's tail (x half1 dead by then)
        stg = [arenaB[:, 19456 + i * 1024: 19456 + (i + 1) * 1024]
               for i in range(4)]
        recip_bf = arenaB[0:1, 23552:24576]
        # attention normalize scratch aliases rope scratch (dead post-rope;
        # DVE program order separates the uses)
        bc_sb = arenaB[:, 17408:19456].bitcast(F32)  # [128, 1024] f32
        zacc = arenaB[:, 16384:17408]                # [128, 1024] bf16
        woslab = wbuf[2][:, 0:4096]    # 8 x [128,512] wo weight tiles
        afbuf = ([arenaB[:, i * 1024:(i + 1) * 1024] for i in range(4)] +
                 [arenaB[:, 12288 + i * 1024: 12288 + (i + 1) * 1024]
                  for i in range(4)])  # wo rhs, 4 slots per half
        y_sb = arenaB[:, 4096:12288].bitcast(F32)       # [128, 4096] f32

        pairs = [ctx.enter_context(nc.psum_tensor(f"pair{i}", [128, 1024], F32))
                 for i in range(4)]

        sems = {n: ctx.enter_context(nc.semaphore(n)) for n in sem_names}

        sch = Sched()
        global LAST_SCHED
        LAST_SCHED = sch
        _schedule(sch, locals())

        with nc.Block() as block:

            @block.sync
            def _(eng):
                sch.run("sync", eng, sems)

            @block.tensor
            def _(eng):
                sch.run("tensor", eng, sems)

            @block.scalar
            def _(eng):
                sch.run("scalar", eng, sems)

            @block.vector
            def _(eng):
                with nc.allow_low_precision(
                        reason="1/Z kept in bf16 for the broadcast matmul"):
                    sch.run("vector", eng, sems)

            @block.gpsimd
            def _(eng):
                sch.run("gpsimd", eng, sems)

    return nc


def _schedule(sch, env):
    def g(n):
        return env[n]

    xT_ext, wq_ext, wk_ext, wv_ext, wo_ext = (
        g("xT_ext"), g("wq_ext"), g("wk_ext"), g("wv_ext"), g("wo_ext"))
    cosd_ext, sins_ext, out_ext = g("cosd_ext"), g("sins_ext"), g("out_ext")
    cc_in, cc_out = g("cc_in"), g("cc_out")
    arenaA, arenaB, wbuf, slab = g("arenaA"), g("arenaB"), g("wbuf"), g("slab")
    qT_sb, kT_sb, v_sb = g("qT_sb"), g("kT_sb"), g("v_sb")
    cosd_sb, sins_sb = g("cosd_sb"), g("sins_sb")
    qsw, t1, t2 = g("qsw"), g("t1"), g("t2")
    stg, recip_sb, bc_sb = g("stg"), g("recip_sb"), g("bc_sb")
    recip_bf = g("recip_bf")
    zacc = g("zacc")
    woslab = g("woslab")
    ones_col, ones_row = g("ones_col"), g("ones_row")
    afbuf, y_sb = g("afbuf"), g("y_sb")
    pairs = g("pairs")
    arenas = [arenaA, arenaB]

    E = sch.emit

    def dma(out_ap, in_ap):
        return lambda eng: eng.dma_start(out=out_ap, in_=in_ap)

    # ---------------- SP: x loads (both halves up front) ----------------
    xsem = {}

    def emit_x_load(half, j):
        name = "xa" if half == 0 else "xb"
        xsem[half] = E("sync", dma(
            arenas[half][:, j * 8192:(j + 1) * 8192]
            .rearrange("p (kb s) -> p kb s", kb=8),
            xT_ext[j * 1024:(j + 1) * 1024, half * 1024:(half + 1) * 1024]
            .rearrange("(kb p) s -> p kb s", p=128)),
            inc=name, amt=16)

    # wb0 first so group 0 can start as soon as the first x quarter lands
    E("vector", lambda eng: eng.memset(ones_col[:], 1.0), inc="dve")
    dve_ones = E("vector", lambda eng: eng.memset(ones_row[:], 1.0), inc="dve")

    # ---------------- projections ----------------
    bank_war = {}      # (pair_idx, colhalf) -> act tick of last reader
    evac_tick = {}     # ("q"/"k", n, half) -> act tick
    wgrp = {}          # qk group idx -> pe tick of its last matmul
    vk_tick = {}       # (half, k) -> pe tick (for slab WAR)
    v_end = {}
    wb_tick = {}
    sl_count = {i: 0 for i in range(4)}

    GL1 = [0, 4, 1, 5, 2, 6, 3, 7]   # half-1 group order: q0 k0 q1 k1 ...

    def emit_qk_weight_dma(gg, engine="sync"):
        half = gg // 8
        gl = GL1[gg - 8] if half == 1 else gg % 8
        t, n = ("q", gl) if gl < 4 else ("k", gl - 4)
        wext = wq_ext if t == "q" else wk_ext
        slot = gg % 3
        deps = []
        if wgrp.get(gg - 3) is not None:
            deps.append(("pe", wgrp[gg - 3]))
        wb_tick[gg] = E(engine, dma(
            wbuf[slot][:].rearrange("p (kb c) -> p kb c", kb=NKB),
            wext[:, n * 128:(n + 1) * 128].rearrange("(kb p) c -> p kb c", p=128)),
            deps=deps, inc=f"wb{slot}", amt=16)

    # startup: sync's DMA ring carries only x half0 + wb0; wb1 and cos/sin
    # go out on the (idle) scalar queue so they don't delay the x ramp
    emit_x_load(0, 0)
    emit_qk_weight_dma(0)
    emit_qk_weight_dma(1, engine="scalar")
    emit_x_load(0, 1)
    emit_x_load(0, 2)
    emit_x_load(0, 3)
    E("scalar", dma(cosd_sb[:], cosd_ext[:]), inc="cs", amt=16)
    CS_ALL = E("scalar", dma(sins_sb[:], sins_ext[:]), inc="cs", amt=16)
    preload_slabs = True

    def emit_one_qk_group(gg, half, gl, pidx):
        t, n = ("q", gl) if gl < 4 else ("k", gl - 4)
        slot = gg % 3
        pair = pairs[pidx]
        xname = "xa" if half == 0 else "xb"
        deps = [(f"wb{slot}", wb_tick[gg])]
        if gg >= 4:
            deps.append((xname, 64))
        else:
            deps.append((xname, 16))
        for chf in range(2):
            if bank_war.get((pidx, chf)) is not None:
                deps.append(("act", bank_war[(pidx, chf)]))
        tick = None
        for k in range(NKB):
            kdeps = ()
            if k == 0:
                kdeps = deps
            elif gg < 4 and k % 8 == 0:
                kdeps = [(xname, 16 * (k // 8 + 1))]
            for sc in range(2):
                last = (k == NKB - 1) and (sc == 1)

                def mm(eng, k=k, sc=sc, pair=pair, slot=slot, half=half):
                    return eng.matmul(
                        pair[:, sc * 512:(sc + 1) * 512],
                        wbuf[slot][:, k * 128:(k + 1) * 128],
                        arenas[half][:, k * 1024 + sc * 512:
                                     k * 1024 + (sc + 1) * 512],
                        start=(k == 0), stop=(k == NKB - 1))

                tick = E("tensor", mm,
                         deps=kdeps if sc == 0 else (),
                         inc="pe" if last else None)
        wgrp[gg] = tick
        dst = qT_sb if t == "q" else kT_sb

        def evac(eng, dst=dst, n=n, half=half, pair=pair):
            return eng.copy(
                dst[:, n * S + half * 1024: n * S + (half + 1) * 1024],
                pair[:, 0:1024])

        a = E("scalar", evac, deps=[("pe", tick)], inc="act")
        bank_war[(pidx, 0)] = a
        bank_war[(pidx, 1)] = a
        evac_tick[(t, n, half)] = a

    def emit_qk_groups0():
        for gl in range(8):
            emit_one_qk_group(gl, 0, gl, gl % 2)
            if gl + 2 <= 7:
                emit_qk_weight_dma(gl + 2)
            if gl in (1, 3, 5, 7):
                emit_x_load(1, (gl - 1) // 2)

    def emit_qk_groups_h1():
        # half-1 q/k groups in q0,k0,q1,k1,... order on psum pairs 2/3,
        # with per-head rope and the first two attention units' QK+exp
        # slices interleaved (their exps hide under the projection PE work)
        for p in range(8):
            gg = 8 + p
            emit_one_qk_group(gg, 1, GL1[p], 2 + p % 2)
            if gg + 3 <= 15:
                emit_qk_weight_dma(gg + 3)
            pass

    def emit_slab(half, k):
        slot = k % 4
        war = vk_tick.get((half, k - 4))
        if war is None and half == 1:
            war = vk_tick.get((0, k + NKB - 4))
        deps = [("pe", war)] if war is not None else []
        sl_count[slot] += 16
        E("sync", dma(
            slab[:, slot * 512:(slot + 1) * 512],
            wv_ext[k * 128:(k + 1) * 128, :]),
          deps=deps, inc=f"sl{slot}", amt=16)

    def emit_v_groups(half):
        tick = None
        for k in range(NKB):
            slot = k % 4
            deps = [(f"sl{slot}", 16 * (half * 8 + k // 4 + 1))]
            if k == 0:
                for pidx in range(4):
                    for chf in range(2):
                        if bank_war.get((pidx, chf)) is not None:
                            deps.append(("act", bank_war[(pidx, chf)]))
            for st in range(8):
                last = st == 7

                def mmv(eng, k=k, st=st, half=half, slot=slot):
                    return eng.matmul(
                        pairs[st // 2][:, (st % 2) * 512:(st % 2 + 1) * 512],
                        arenas[half][:, k * 1024 + st * 128:
                                     k * 1024 + st * 128 + 128],
                        slab[:, slot * 512:(slot + 1) * 512],
                        start=(k == 0), stop=(k == NKB - 1))

                tick = E("tensor", mmv, deps=deps if st == 0 else (),
                         inc="pe" if last else None)
            vk_tick[(half, k)] = tick
            if k + 4 < NKB:
                emit_slab(half, k + 4)
            if half == 0 and k < 2:
                emit_qk_weight_dma(8 + k)
            if half == 0 and k == 2:
                emit_qk_weight_dma(10)
        v_end[half] = tick
        for st in range(8):
            stg_idx = half * 8 + st

            def evacv(eng, stg_idx=stg_idx, st=st):
                return eng.copy(
                    v_sb[:, stg_idx * 512:(stg_idx + 1) * 512],
                    pairs[st // 2][:, (st % 2) * 512:(st % 2 + 1) * 512])

            evdeps = [("pe", v_end[half])]
            if half == 1:
                evdeps.append(("dve", rope_last))  # v_sb rope-scratch WAR
            a = E("scalar", evacv, deps=evdeps, inc="act")
            bank_war[(st // 2, st % 2)] = a

    # ---------------- RoPE (in-place, v_sb tail scratch) ----------------
    swp = 0
    prev_sw = None
    rope_last = None
    rope_done = {}
    rp_t1 = v_sb[:, 4096:6144]   # v(h1) region: free until v(h1) evacs

    def emit_rope(t, tsb, h):
        nonlocal swp, prev_sw, rope_last
        c0 = h * S
        d0 = [("act", evac_tick[(t, h, 0)]), ("act", evac_tick[(t, h, 1)])]
        dsw = d0 + ([("dve", prev_sw)] if prev_sw is not None else [])
        swp = E("scalar", dma(v_sb[0:64, 6144:8192],
                              tsb[64:128, c0:c0 + S]),
                deps=dsw, inc="swp", amt=16)
        swp = E("scalar", dma(v_sb[64:128, 6144:8192],
                              tsb[0:64, c0:c0 + S]), inc="swp", amt=16)

        def f_t1(eng, tsb=tsb, c0=c0):
            return eng.tensor_mul(rp_t1, tsb[:, c0:c0 + S], cosd_sb[:])

        E("vector", f_t1, deps=d0 + [("cs", CS_ALL)], inc="dve")

        def f_t2(eng, tsb=tsb, c0=c0):
            return eng.tensor_mul(tsb[:, c0:c0 + S], v_sb[:, 6144:8192],
                                  sins_sb[:])

        prev_sw = E("vector", f_t2, deps=[("swp", swp)], inc="dve")

        def f_add(eng, tsb=tsb, c0=c0):
            return eng.tensor_add(tsb[:, c0:c0 + S], tsb[:, c0:c0 + S],
                                  rp_t1)

        rope_last = rope_done[(t, h)] = E("vector", f_add, inc="dve")

    # ------------- attention: 8 half-units (head-major) -------------
    # dunit d = h*2 + qp covers head h, s-half qp (two sq quarters).
    # expT slab (d%2) = arenaA[:, (d%2)*16384 : +16384] as [16 sk][1024].
    # AllGather is per local head (4 gathers): gather(h) fires as soon as
    # units 2h, 2h+1 have stored, so gathers h0-h2 hide under attention.
    # wo kb enumerates (head, core): kb = h*8 + c -> gathered rows
    # cc_out[h][c*128:...], weight rows wo_ext[(c*4 + h)*128:...].
    wo_kb_tick = {}
    af = {i: 0 for i in range(8)}
    wl_count = {i: 0 for i in range(8)}
    af_loaded = set()
    wl_loaded = set()

    def emit_afbuf_load(half, kb, engine="sync"):
        if (half, kb) in af_loaded:
            return
        af_loaded.add((half, kb))
        h, c = kb // 8, kb % 8
        aslot = half * 4 + kb % 4
        war = wo_kb_tick.get((half, kb - 4))
        deps = [("cc", h + 1)]
        if war is not None:
            deps.append(("pe", war))
        af[aslot] += 16
        E(engine, dma(
            afbuf[aslot],
            cc_out[h][c * 128:(c + 1) * 128,
                      half * 1024:(half + 1) * 1024]),
          deps=deps, inc=f"af{aslot}", amt=16)

    def emit_woslab_load(half, kb, engine="sync"):
        if (half, kb) in wl_loaded:
            return
        wl_loaded.add((half, kb))
        h, c = kb // 8, kb % 8
        wslot = kb % 8
        sdeps = [("pe", wgrp[14])]   # wbuf[2] WAR (last qk reader)
        swar = wo_kb_tick.get((half, kb - 8))
        if swar is None and half == 1:
            swar = wo_kb_tick.get((0, kb + NKB - 8))
        if swar is not None:
            sdeps.append(("pe", swar))
        wl_count[wslot] += 16
        wrow = (c * NH + h) * 128
        E(engine, dma(
            woslab[:, wslot * 512:(wslot + 1) * 512],
            wo_ext[wrow:wrow + 128, :]),
          deps=sdeps, inc=f"wl{wslot}", amt=16)

    def emit_wo_loads(half, kb, engine="sync"):
        emit_afbuf_load(half, kb, engine)
        emit_woslab_load(half, kb, engine)

    exp_last = {}
    pv_read_end = {}
    state = {"stt": None, "recip": None, "bc": None, "bcast": None,
             "zmm": None, "adds": {}}
    stg_store = {}
    store_tick = {}
    ps_o_pair = {}
    ast = {0: 0, 1: 0, 2: 0, 3: 0}
    pending_zr = []
    pending_bc = []
    pending_adds = []
    adds_l1 = {}

    def finish_unit(d, bcast_tick):
        h, qp = d // 2, d % 2
        state["bc"] = E(
            "vector",
            lambda eng: eng.tensor_copy(bc_sb[:], pairs[3][:, 0:1024]),
            deps=[("pe", bcast_tick)], inc="dve")
        slot = d % 4
        sdeps = []
        if slot in stg_store:
            sdeps.append(stg_store[slot])

        def f_stt(eng, slot=slot, d=d):
            return eng.scalar_tensor_tensor(
                stg[slot][:], pairs[ps_o_pair[d]][:, 0:1024], 1.0, bc_sb[:],
                ALU.mult, ALU.mult)

        state["stt"] = E("vector", f_stt, deps=sdeps, inc="dve")

        sem = f"ast{slot}"
        ast[slot] += 16
        E("sync", dma(
            cc_in[h * 128:(h + 1) * 128, qp * 1024:(qp + 1) * 1024],
            stg[slot][:]),
            deps=[("dve", state["stt"])], inc=sem, amt=16)
        stg_store[slot] = (sem, ast[slot])
        store_tick[d] = (sem, ast[slot])

    def make_zr(d):
        def emit_zr():
            dps = [("dve", state["adds"][d]), ("dve", dve_ones)]
            if state["recip"] is not None:
                dps.append(("dve", state["recip"]))  # ps_z WAR
            if state["bc"] is not None:
                dps.append(("dve", state["bc"]))  # pairs[3] WAR vs bc copy
            for chf in range(2):
                bw = bank_war.get((3, chf))
                if bw is not None:
                    dps.append(("act", bw))  # pairs[3] WAR vs v(h1) evacs
            # Z = ones.T @ partials: contract the remaining 4 partial
            # slices on the PE (8 accumulating mms) instead of more DVE adds
            zmm = None
            for ch in range(4):
                for zc in range(2):
                    def fz(eng, ch=ch, zc=zc):
                        return eng.matmul(
                            pairs[3][0:1, zc * 512:(zc + 1) * 512], ones_col[:],
                            wbuf[0][:, ch * 1024 + zc * 512:
                                    ch * 1024 + (zc + 1) * 512],
                            start=(ch == 0), stop=(ch == 3))
                    last = ch == 3 and zc == 1
                    zmm = E("tensor", fz,
                            deps=dps if (ch == 0 and zc == 0) else (),
                            inc="pe" if last else None)
            state["zmm"] = zmm
            rdeps = [("pe", zmm)]
            if state["bcast"] is not None:
                rdeps.append(("pe", state["bcast"]))  # recip_bf WAR
            state["recip"] = E(
                "vector",
                lambda eng: eng.reciprocal(recip_bf[:], pairs[3][0:1, 0:1024]),
                deps=rdeps, inc="dve")
        return emit_zr

    def make_bcast(d):
        def emit_bcast():
            dps = [("dve", state["recip"])]
            if state["bc"] is not None:
                dps.append(("dve", state["bc"]))
            for chf in range(2):
                bw = bank_war.get((3, chf))
                if bw is not None:
                    dps.append(("act", bw))  # pairs[3] WAR vs v(h1) evacs
            bt = None
            for zc in range(2):
                def fb(eng, zc=zc):
                    return eng.matmul(
                        pairs[3][:, zc * 512:(zc + 1) * 512], ones_row[:],
                        recip_bf[:, zc * 512:(zc + 1) * 512],
                        start=True, stop=True)
                bt = E("tensor", fb, deps=dps if zc == 0 else (),
                       inc="pe" if zc == 1 else None)
            state["bcast"] = bt
            finish_unit(d, bt)
        return emit_bcast

    def emit_pass1(d, sks=None, filler=None):
        h, qp = d // 2, d % 2
        base = (d % 2) * 16384
        fill_ticks = {}
        if sks is None:
            sks = range(16)
        for sk in sks:
            pidx = sk % 2
            pair = pairs[pidx]
            deps = [("dve", rope_done[("q", h)]), ("dve", rope_done[("k", h)])]
            for chf in range(2):
                if bank_war.get((pidx, chf)) is not None:
                    deps.append(("act", bank_war[(pidx, chf)]))
            tick = None
            for qi in range(2):

                def mm1(eng, pair=pair, h=h, sk=sk, qp=qp, qi=qi):
                    return eng.matmul(
                        pair[:, qi * 512:(qi + 1) * 512],
                        kT_sb[:, h * S + sk * 128: h * S + sk * 128 + 128],
                        qT_sb[:, h * S + qp * 1024 + qi * 512:
                              h * S + qp * 1024 + (qi + 1) * 512],
                        start=True, stop=True)

                tick = E("tensor", mm1, deps=deps if qi == 0 else (),
                         inc="pe" if qi == 1 else None)

            if sk == 0 and pending_zr:
                pending_zr.pop(0)()
            if sk == 2 and pending_adds:
                pending_adds.pop(0)()
            if sk == 14 and pending_bc:
                # sk14 (not 8): the interleaved PV(d-2) finishes at sk11, and
                # stt(d-2) must read its psum only after that
                pending_bc.pop(0)()

            # interleave PV(d-2) slices between QK pairs: they fill the
            # exp-lockstep gaps on the PE. Emitted BEFORE this sk's exp so
            # the exp's per-slice slab WAR can reference the PV tick.
            if filler is not None:
                if sk < 12:
                    fill_ticks[sk] = filler(sk)
                elif sk == 12:
                    for s2 in range(12, 16):
                        fill_ticks[s2] = filler(s2)

            edeps = [("pe", tick)]
            if d >= 2 and sk == 0:
                edeps.append(("dve", adds_l1[d - 2]))
            if filler is not None:
                # exp(d, sk) overwrites the slab slice PV(d-2, sk) reads
                edeps.append(("pe", fill_ticks[sk]))
            if d < 2 and sk == 0:
                edeps.append(("pe", P_H0_END))  # arenaA WAR vs half0 x

            def f_exp(eng, pair=pair, base=base, sk=sk):
                return eng.activation(
                    arenaA[:, base + sk * 1024: base + (sk + 1) * 1024],
                    pair[:, 0:1024], AF.Exp, scale=SCALE)

            a = E("scalar", f_exp, deps=edeps, inc="act")
            exp_last[d] = a
            bank_war[(pidx, 0)] = a
            bank_war[(pidx, 1)] = a
        if 15 in sks:
            pending_adds.append(make_adds(d))

    def begin_pv(d, opair=2):
        # slice-granular PV emitter so PV(d) can interleave into the NEXT
        # unit's QK stream, filling the exp-lockstep gaps with real work
        h = d // 2
        base = (d % 2) * 16384
        deps = [("act", exp_last[d])]
        if opair == 2 and state["stt"] is not None:
            deps.append(("dve", state["stt"]))
        # pair WAR + v_sb RAW vs the v(h1) evacs: units 0/1's exps precede
        # the v evacs on the act queue, so program order no longer covers
        # it. (3,1) is the last v evac; stale-but-harmless for later units.
        bw = bank_war.get((3, 1))
        if bw is not None:
            deps.append(("act", bw))
        for chf in range(2):
            bw = bank_war.get((opair, chf))
            if bw is not None:
                deps.append(("act", bw))
        ps_o_pair[d] = opair
        pending_zr.append(make_zr(d))
        pending_bc.append(make_bcast(d))

        def emit_pv_slice(sk):
            tick = None
            for qi in range(2):

                def mpv(eng, sk=sk, h=h, base=base, qi=qi, opair=opair):
                    return eng.matmul(
                        pairs[opair][:, qi * 512:(qi + 1) * 512],
                        v_sb[:, sk * 512 + h * 128: sk * 512 + h * 128 + 128],
                        arenaA[:, base + sk * 1024 + qi * 512:
                               base + sk * 1024 + (qi + 1) * 512],
                        start=(sk == 0), stop=(sk == 15))

                tick = E("tensor", mpv,
                         deps=deps if (sk == 0 and qi == 0) else (),
                         inc="pe" if qi == 1 else None)
            if sk == 15:
                pv_read_end[d] = tick
            return tick
        return emit_pv_slice

    def emit_pass2(d, opair=2):
        pv = begin_pv(d, opair)
        for sk in range(16):
            pv(sk)

    def make_adds(d):
        # reduce 16 expT slices to 4 partials in wbuf[0] (dead after the
        # last qk group; the PE's zmm contracts the rest). Popped into the
        # NEXT pass1 window so the DVE queue never blocks the z-chain.
        base = (d % 2) * 16384

        def emit_adds():
            adeps = [("act", exp_last[d]), ("pe", wgrp[15])]
            if state["zmm"] is not None:
                adeps.append(("pe", state["zmm"]))  # scratch WAR vs zmm reads

            def fa(eng, q, acc, base=base):
                sl = arenaA[:, base + q * 4096:base + (q + 1) * 4096]
                if not acc:
                    return eng.tensor_add(
                        wbuf[0][:, 0:4096],
                        sl, arenaA[:, base + 4096 * (q + 1):
                                   base + 4096 * (q + 2)])
                return eng.tensor_add(wbuf[0][:, 0:4096],
                                      wbuf[0][:, 0:4096], sl)

            E("vector", lambda eng: fa(eng, 0, False), deps=adeps)
            E("vector", lambda eng: fa(eng, 2, True))
            tick = E("vector", lambda eng: fa(eng, 3, True), inc="dve")
            adds_l1[d] = state["adds"][d] = tick
        return emit_adds

    # PE order: qk(h0), v(h0), qk(h1)+rope+pass1(0,1), v(h1), attention
    for k in range(4):
        emit_slab(0, k)
    emit_qk_groups0()
    emit_v_groups(0)
    P_H0_END = v_end[0]
    for k in range(4):
        emit_slab(1, k)
    emit_qk_groups_h1()
    # ropes after all h1 groups: keeps the scalar queue's evac stream
    # unblocked (rope swap DMAs would HOL-block evacs mid-projection);
    # they drain during v(h1), well before pass1 needs them
    for hh in range(NH):
        emit_rope("q", qT_sb, hh)
        emit_rope("k", kT_sb, hh)
    emit_v_groups(1)

    emit_pass1(0)
    emit_pass1(1)
    for d in range(2, 8):
        emit_pass1(d, filler=begin_pv(d - 2))
        if d == 3:
            # woslab tiles for head 0 prefetch during early attention
            # (wbuf[2] WAR only — no collective dependency)
            for kb_pre in range(8):
                emit_woslab_load(0, kb_pre)
    # afbuf preloads sit on the sync queue BEFORE the last units' cc_in
    # stores; their cc(1) dep is satisfied mid-attention so they stream in
    # well before the wo matmuls need them
    for kb_pre in range(4):
        emit_afbuf_load(0, kb_pre)
    emit_pass2(6)             # PV(6); queues zr6/bc6
    pending_zr.pop(0)()       # zmm(6)+recip(6): reads ztree(6) before L1(7)
    pending_adds.pop(0)()     # tree(7), gated on zmm(6) via ztree WAR
    emit_pass2(7, opair=0)    # PV(7) -> pair0, overlaps unit 6's chain
    pending_bc.pop(0)()       # bcast(6)+stt(6)
    pending_zr.pop(0)()       # zmm(7)+recip(7)
    pending_bc.pop(0)()       # bcast(7)+stt(7)

    for h in range(NH):

        def f_ag(eng, h=h):
            return eng.collective_compute(
                "AllGather", ALU.bypass,
                replica_groups=[list(range(N_CORES))],
                ins=[cc_in[h * 128:(h + 1) * 128, :].opt()],
                outs=[cc_out[h][:].opt()])

        E("gpsimd", f_ag,
          deps=[store_tick[2 * h], store_tick[2 * h + 1]],
          inc="cc")

    # ---------------- wo projection ----------------
    y_evac_by_dout = {}
    for half in range(2):
        for kb in range(NKB):
            slot = kb % 4
            aslot = half * 4 + slot
            wslot = kb % 8
            emit_wo_loads(half, kb)

            mmdeps = [(f"af{aslot}", af[aslot]), (f"wl{wslot}", wl_count[wslot])]
            if kb == 0 and half == 0:
                mmdeps.append(("act", exp_last[7]))
                mmdeps.append(("dve", state["stt"]))
                mmdeps.append(("dve", state["recip"]))
            tick = None
            for dout in range(4):
                for c2 in range(2):
                    dd = mmdeps if (dout == 0 and c2 == 0) else []
                    if kb == 0 and half == 1 and c2 == 0:
                        # pairs[dout] WAR: only needs half-0's evacs of
                        # this dout, not the full evac+store tail
                        dd = list(dd) + [("act", y_evac_by_dout[dout])]

                    def mmo(eng, kb=kb, dout=dout, c2=c2,
                            aslot=aslot, wslot=wslot):
                        return eng.matmul(
                            pairs[dout][:, c2 * 512:(c2 + 1) * 512],
                            woslab[:, wslot * 512 + dout * 128:
                                   wslot * 512 + dout * 128 + 128],
                            afbuf[aslot][:, c2 * 512:(c2 + 1) * 512],
                            start=(kb == 0), stop=(kb == NKB - 1))

                    tick = E("tensor", mmo,
                             deps=dd,
                             inc="pe" if (dout == 3 and c2 == 1) else None)
            wo_kb_tick[(half, kb)] = tick

        wo_end = wo_kb_tick[(half, NKB - 1)]
        if half == 0:
            for kb_pre in range(4):
                emit_wo_loads(1, kb_pre, engine="scalar")
        for c2 in range(2):
            for dout in range(4):
                deps = [("pe", wo_end)]
                if half == 1:
                    deps.append(("yst", 16 * (c2 * 4 + dout + 1)))

                def evy(eng, dout=dout, c2=c2):
                    return eng.copy(
                        y_sb[:, (dout * 2 + c2) * 512:(dout * 2 + c2 + 1) * 512],
                        pairs[dout][:, c2 * 512:(c2 + 1) * 512])

                y_evac_last = E("scalar", evy, deps=deps, inc="act")
                if c2 == 1:
                    y_evac_by_dout[dout] = y_evac_last

                cbase = half * 1024 + c2 * 512
                E("sync", dma(
                    out_ext[dout * 128:(dout + 1) * 128, cbase:cbase + 512],
                    y_sb[:, (dout * 2 + c2) * 512:(dout * 2 + c2 + 1) * 512]),
                    deps=[("act", y_evac_last)], inc="yst", amt=16)

    E("sync", lambda eng: None, deps=[("yst", 256)])


# ======================= host side =======================

_NC_CACHE = None


def _get_nc():
    global _NC_CACHE
    if _NC_CACHE is None:
        _NC_CACHE = build()
    return _NC_CACHE


def _prep_inputs(x, freqs_cos, freqs_sin, wq, wk, wv, wo):
    bf = ml_dtypes.bfloat16
    x2 = np.asarray(x, np.float32).reshape(S, D)
    xT = np.ascontiguousarray(x2.T).astype(bf)
    perm = np.concatenate([np.arange(0, HD, 2), np.arange(1, HD, 2)])
    cos = np.asarray(freqs_cos, np.float32)
    sin = np.asarray(freqs_sin, np.float32)
    cosd = np.concatenate([cos.T, cos.T], axis=0).astype(bf)
    sins = np.concatenate([-sin.T, sin.T], axis=0).astype(bf)

    in_maps = []
    for c in range(N_CORES):
        cols_qk = np.concatenate([c * NL + h * HD + perm for h in range(NH)])
        cols_n = np.arange(c * NL, (c + 1) * NL)
        in_maps.append({
            "xT": xT,
            "wq": np.ascontiguousarray(np.asarray(wq, np.float32)[:, cols_qk]).astype(bf),
            "wk": np.ascontiguousarray(np.asarray(wk, np.float32)[:, cols_qk]).astype(bf),
            "wv": np.ascontiguousarray(np.asarray(wv, np.float32)[:, cols_n]).astype(bf),
            "wo": np.ascontiguousarray(np.asarray(wo, np.float32)[:, cols_n]).astype(bf),
            "cosd": cosd,
            "sins": sins,
        })
    return in_maps


def run(inputs, trace=False, **kw):
    nc = _get_nc()
    in_maps = _prep_inputs(**inputs)
    res = run_bass_kernel_spmd(nc, in_maps, core_ids=list(range(N_CORES)),
                               trace=trace, **kw)
    yT = np.concatenate([np.asarray(res.results[c]["out"], np.float32)
                         for c in range(N_CORES)], axis=0)
    out = np.ascontiguousarray(yT.T).reshape(1, S, D).astype(np.float32)
    return out, res


def kernel(x, freqs_cos, freqs_sin, wq, wk, wv, wo):
    out, _ = run(dict(x=x, freqs_cos=freqs_cos, freqs_sin=freqs_sin,
                      wq=wq, wk=wk, wv=wv, wo=wo))
    return out

